# revision 16
# baseline (speedup 1.0000x reference)
"""Trainium2 Bass kernel for the double-Mamba block (nn_DoubleConv).

Sharding: 8 cores = 4 batches x 2 sequence halves. Each core processes
LC = 2048 + OV columns of its batch element; OV columns are burn-in
(delta >= 0.54 => per-step state decay <= e^-0.54, so OV/2 columns per
mamba layer push the truncation error below fp32 noise).

Layout: features on partitions, time on the free axis.
 - causal depthwise conv folded into in_proj: 4 accumulating PE matmuls
   with shifted rhs, lhsT_j = (conv_w[:, j] * W_in_xc).T
 - dA_n = Exp(delta * A[:, n]) on ScalarE (per-partition scale AP)
 - h_n via VectorE tensor_tensor_scan (fp32 state)
 - B/C rows broadcast across partitions via DRAM round-trip DMAs
 - y = sum_n C_n*h_n via PE identity-matmul PSUM accumulation
 - LayerNorm over the feature(partition) axis via ones/128 matmuls
"""
import numpy as np
from contextlib import ExitStack

import concourse.bass as bass
import concourse.bacc as bacc
import concourse.mybir as mybir
import concourse.tile as tile
from concourse.bass_utils import run_bass_kernel_spmd

F32 = mybir.dt.float32
F16 = mybir.dt.float16
AF = mybir.ActivationFunctionType
OP = mybir.AluOpType

D_STATE = 16
D_CONV = 4
B, L, IN_C, OUT_C = 4, 4096, 64, 128
OV = 128                      # burn-in columns (covers both layers)
LC = 2048 + OV                # per-core columns
LPAD = LC + 3                 # conv left-pad
BLK = 512                     # PSUM block


def _blocks(n, bs=BLK):
    return [(s, min(s + bs, n)) for s in range(0, n, bs)]


class _DmaRR:
    """DMA issue router: bulk broadcasts alternate Sync/GpSimd queues so
    the Scalar sequencer stays free for ACTIVATE dispatch."""

    def __init__(self, nc):
        self.nc = nc
        self.i = 0

    def __call__(self, out, in_):
        e = [self.nc.sync, self.nc.gpsimd][self.i % 2]
        self.i += 1
        return e.dma_start(out, in_)

    def wload(self, out, in_):
        return self.nc.scalar.dma_start(out, in_)


def _layer_norm(nc, pools, dma, lnrows_dram, row_base,
                h_raw, g_col, b_col, out_tile, out_off=0, col0=0, ncols=LC):
    """LN over the partition axis of h_raw[:, col0:col0+ncols] (f16, SBUF).
    Writes f16 into out_tile[:, out_off+col0 : out_off+col0+ncols]."""
    sb, mmp, vec = pools['sb'], pools['mm'], pools['vec']
    ones_over = pools['ones128']     # [128, 1] f16 of 1/128
    c1 = col0 + ncols
    h_sq = sb.tile([128, LC], F16, tag="lnsq", name="lnsq")
    nc.scalar.activation(h_sq[:, col0:c1], h_raw[:, col0:c1], AF.Square)
    vA = vec.tile([128, LC], F32, tag="vA", name="vA")
    vB = vec.tile([128, LC], F32, tag="vB", name="vB")
    msq, var, mu = vA[0:1, :], vA[32:33, :], vA[64:65, :]
    mu2, s_row = vB[0:1, :], vB[64:65, :]
    mus_row = vA[96:97, :]
    for (s, e) in _blocks(ncols):
        s, e = s + col0, e + col0
        p1 = mmp.tile([1, BLK], F32, tag="mm", name="mm")
        nc.tensor.matmul(p1[:, :e - s], ones_over[:], h_raw[:, s:e],
                         start=True, stop=True)
        nc.scalar.activation(mu[:, s:e], p1[:, :e - s], AF.Copy)
        p2 = mmp.tile([1, BLK], F32, tag="mm", name="mm")
        nc.tensor.matmul(p2[:, :e - s], ones_over[:], h_sq[:, s:e],
                         start=True, stop=True)
        nc.scalar.activation(msq[:, s:e], p2[:, :e - s], AF.Copy)
    nc.scalar.activation(mu2[:, col0:c1], mu[:, col0:c1], AF.Square)
    nc.vector.tensor_tensor(out=var[:, col0:c1], in0=msq[:, col0:c1],
                            in1=mu2[:, col0:c1], op=OP.subtract)
    nc.scalar.activation(s_row[:, col0:c1], var[:, col0:c1],
                         AF.Abs_reciprocal_sqrt, bias=pools['eps1'][:1, :])
    nc.vector.tensor_tensor(out=mus_row[:, col0:c1], in0=mu[:, col0:c1],
                            in1=s_row[:, col0:c1], op=OP.mult)
    # f32 -> f16 cast happens inside the gpsimd software-DGE DMA
    nc.gpsimd.dma_start(out=lnrows_dram.ap()[row_base:row_base + 1, col0:c1],
                        in_=s_row[:, col0:c1])
    nc.gpsimd.dma_start(
        out=lnrows_dram.ap()[row_base + 1:row_base + 2, col0:c1],
        in_=mus_row[:, col0:c1])
    s_bc = sb.tile([128, LC], F16, tag="lnbc0", name="lnbc0")
    mus_bc = sb.tile([128, LC], F16, tag="lnbc1", name="lnbc1")
    dma(s_bc[:, col0:c1], lnrows_dram.ap()[row_base:row_base + 1, col0:c1]
        .broadcast_to((128, ncols)))
    dma(mus_bc[:, col0:c1],
        lnrows_dram.ap()[row_base + 1:row_base + 2, col0:c1]
        .broadcast_to((128, ncols)))
    # out = ((h*s_bc) - mus_bc)*g + b
    t1 = sb.tile([128, LC], F16, tag="lnt1", name="lnt1")
    nc.vector.tensor_tensor(out=t1[:, col0:c1], in0=h_raw[:, col0:c1],
                            in1=s_bc[:, col0:c1], op=OP.mult)
    t2 = sb.tile([128, LC], F16, tag="lnsq", name="lnsq")
    nc.vector.tensor_tensor(out=t2[:, col0:c1], in0=t1[:, col0:c1],
                            in1=mus_bc[:, col0:c1], op=OP.subtract)
    nc.vector.tensor_scalar(out=out_tile[:, out_off + col0:out_off + c1],
                            in0=t2[:, col0:c1], scalar1=g_col[:],
                            scalar2=b_col[:], op0=OP.mult, op1=OP.add)


def _mamba(nc, pools, dma, W, lay, xin, xin_off, di, dtr,
           brow_dram, crow_dram, n_wo_grp, lnrows_dram, ln_row, g_col,
           b_col, out_norm, out_norm_off):
    """One mamba layer, processed in 2 time-chunks so chunk-1 prep overlaps
    chunk-0 scans. Includes out_proj and LayerNorm. Writes normalized f16
    into out_norm[:, out_norm_off : out_norm_off+LC]."""
    sb, mmp, yp = pools['sb'], pools['mm'], pools['yacc']
    n_grp = di // 128
    HC = LC // 2
    hlast = [sb.tile([128, 16], F16, tag=f"hlast_{g}", name="hlast")
             for g in range(n_grp)]
    o_raw = sb.tile([128, LC], F16, tag="rawbuf", name="rawbuf")
    for c in range(2):
        c0 = c * HC
        cols = [(c0 + s_, c0 + e_) for (s_, e_) in _blocks(HC)]
        xc2 = [sb.tile([128, LC], F16, tag=f"xc_{g}", name="xc")
               for g in range(n_grp)] if c == 0 else _mamba.xc2
        sres = [sb.tile([128, LC], F16, tag=f"sres_{g}", name="sres")
                for g in range(n_grp)] if c == 0 else _mamba.sres
        if c == 0:
            _mamba.xc2, _mamba.sres = xc2, sres
        for g in range(n_grp):
            for (s, e) in cols:
                mm = mmp.tile([128, BLK], F32, tag="mm", name="mm")
                for j in range(D_CONV):
                    nc.tensor.matmul(
                        mm[:, :e - s], W[f'Mj{lay}_{j}_{g}'][:],
                        xin[:, xin_off - 3 + j + s: xin_off - 3 + j + e],
                        start=(j == 0), stop=(j == D_CONV - 1))
                nc.scalar.activation(xc2[g][:, s:e], mm[:, :e - s], AF.Silu,
                                     bias=W[f'convb{lay}_{g}'][:])
                mm2 = mmp.tile([128, BLK], F32, tag="mm", name="mm")
                nc.tensor.matmul(mm2[:, :e - s], W[f'Wres{lay}_{g}'][:],
                                 xin[:, xin_off + s: xin_off + e],
                                 start=True, stop=True)
                nc.scalar.activation(sres[g][:, s:e], mm2[:, :e - s], AF.Silu)
        nxd = dtr + 32
        xdbl16 = sb.tile([nxd, LC], F16, tag="xdbl", name="xdbl") \
            if c == 0 else _mamba.xdbl
        dt32 = sb.tile([dtr, LC], F32, tag="dt32", name="dt32") \
            if c == 0 else _mamba.dt32
        if c == 0:
            _mamba.xdbl, _mamba.dt32 = xdbl16, dt32
        for (s, e) in cols:
            mm = mmp.tile([nxd, BLK], F32, tag="mm", name="mm")
            for g in range(n_grp):
                nc.tensor.matmul(mm[:, :e - s], W[f'xpT{lay}_{g}'][:],
                                 xc2[g][:, s:e],
                                 start=(g == 0), stop=(g == n_grp - 1))
            nc.scalar.activation(xdbl16[:, s:e], mm[:, :e - s], AF.Copy)
            nc.scalar.activation(dt32[:, s:e], mm[:dtr, :e - s], AF.Copy)
        dma(brow_dram.ap()[:, c0:c0 + HC], xdbl16[dtr:dtr + 16, c0:c0 + HC])
        dma(crow_dram.ap()[:, c0:c0 + HC],
            xdbl16[dtr + 16:dtr + 32, c0:c0 + HC])
        for g in range(n_grp):
            # delta' = ln(sigmoid(-(pre + dt_b))) = -softplus(pre + dt_b)
            delta = sb.tile([128, HC], F32, tag="delta", name="delta")
            sigout = sb.tile([128, HC], F32, tag="sigout", name="sigout")
            for (s, e) in cols:
                mm = mmp.tile([128, BLK], F32, tag="mm", name="mm")
                nc.tensor.matmul(mm[:, :e - s], W[f'dtwT{lay}_{g}'][:],
                                 dt32[:, s:e], start=True, stop=True)
                nc.scalar.activation(sigout[:, s - c0:e - c0], mm[:, :e - s],
                                     AF.Sigmoid, bias=W[f'dtbn{lay}_{g}'][:],
                                     scale=-1.0)
            nc.scalar.activation(delta[:], sigout[:], AF.Ln)
            w16 = sb.tile([128, HC], F16, tag="w16", name="w16")
            nc.vector.tensor_tensor(out=w16[:], in0=delta[:],
                                    in1=xc2[g][:, c0:c0 + HC], op=OP.mult)
            ytiles = [yp.tile([128, BLK], F32, tag="yacc", name="yacc")
                      for _ in cols]

            def _emit_q(n, h, c_bc):
                q = sb.tile([128, HC], F16, tag="q", name="q")
                nc.vector.tensor_tensor(out=q[:], in0=h[:], in1=c_bc[:],
                                        op=OP.mult)
                for bi, (s, e) in enumerate(cols):
                    nc.tensor.matmul(ytiles[bi][:, :e - s], pools['ident'][:],
                                     q[:, s - c0:e - c0],
                                     start=(n == 0), stop=(n == 15))

            prev = None
            for n in range(16):
                dA = sb.tile([128, HC], F32, tag="dA", name="dA")
                nc.scalar.activation(dA[:], delta[:], AF.Exp,
                                     scale=W[f'A{lay}_{g}'][:, n:n + 1])
                b_bc = sb.tile([128, HC], F16, tag="bbc", name="bbc")
                dma(b_bc[:], brow_dram.ap()[n:n + 1, c0:c0 + HC]
                    .broadcast_to((128, HC)))
                c_bc = sb.tile([128, HC], F16, tag="cbc", name="cbc")
                dma(c_bc[:], crow_dram.ap()[n:n + 1, c0:c0 + HC]
                    .broadcast_to((128, HC)))
                dBu = sb.tile([128, HC], F16, tag="dbu", name="dbu")
                nc.vector.tensor_tensor(out=dBu[:], in0=w16[:], in1=b_bc[:],
                                        op=OP.mult)
                if prev is not None:
                    _emit_q(*prev)
                h = sb.tile([128, HC], F16, tag="h", name="h")
                init = 0.0 if c == 0 else hlast[g][:, n:n + 1]
                nc.vector.tensor_tensor_scan(h[:], dA[:], dBu[:], init,
                                             OP.mult, OP.add)
                if c == 0:
                    nc.vector.tensor_copy(hlast[g][:, n:n + 1], h[:, -1:])
                prev = (n, h, c_bc)
            _emit_q(*prev)
            m_raw = sb.tile([128, HC], F16, tag=f"mraw_{g}", name="mraw") \
                if True else None
            for bi, (s, e) in enumerate(cols):
                t1 = sb.tile([128, BLK], F32, tag="gt1", name="gt1")
                nc.vector.scalar_tensor_tensor(
                    t1[:, :e - s], xc2[g][:, s:e], W[f'D{lay}_{g}'][:],
                    ytiles[bi][:, :e - s], OP.mult, OP.add)
                nc.vector.tensor_tensor(out=m_raw[:, s - c0:e - c0],
                                        in0=t1[:, :e - s],
                                        in1=sres[g][:, s:e], op=OP.mult)
            if g == 0:
                _mamba.mraws = []
            _mamba.mraws.append(m_raw)
        # out_proj over groups, then LN on this chunk
        for (s, e) in cols:
            mm = mmp.tile([128, BLK], F32, tag="mm", name="mm")
            for g in range(n_grp):
                nc.tensor.matmul(mm[:, :e - s], W[f'{n_wo_grp}{lay}_{g}'][:],
                                 _mamba.mraws[g][:, s - c0:e - c0],
                                 start=(g == 0), stop=(g == n_grp - 1))
            nc.scalar.activation(o_raw[:, s:e], mm[:, :e - s], AF.Copy)
        _layer_norm(nc, pools, dma, lnrows_dram, ln_row,
                    o_raw, g_col, b_col, out_norm,
                    out_off=out_norm_off, col0=c0, ncols=HC)


def build_nc():
    nc = bacc.Bacc("TRN2", target_bir_lowering=False, debug=False)
    dram_w = {}

    def reg(name, shape, dt):
        dram_w[name] = nc.dram_tensor(name, list(shape), dt,
                                      kind="ExternalInput")

    x_d = nc.dram_tensor("x_t", [IN_C, LPAD], F16, kind="ExternalInput")
    out_d = nc.dram_tensor("out", [128, 2048], F32, kind="ExternalOutput")
    for lay, (dm, di, dtr) in {1: (IN_C, 128, 4), 2: (OUT_C, 256, 8)}.items():
        for g in range(di // 128):
            for j in range(D_CONV):
                reg(f'Mj{lay}_{j}_{g}', [dm, 128], F16)
            reg(f'Wres{lay}_{g}', [dm, 128], F16)
            reg(f'convb{lay}_{g}', [128, 1], F32)
            reg(f'xpT{lay}_{g}', [128, dtr + 32], F16)
            reg(f'dtwT{lay}_{g}', [dtr, 128], F32)
            reg(f'dtbn{lay}_{g}', [128, 1], F32)
            reg(f'A{lay}_{g}', [128, 16], F32)
            reg(f'D{lay}_{g}', [128, 1], F32)
            reg(f'WoT{lay}_{g}', [128, 128], F16)
        reg(f'ln{lay}_g', [128, 1], F32)
        reg(f'ln{lay}_b', [128, 1], F32)
    reg('WlinT', [IN_C, 128], F16)
    reg('linb', [128, 1], F32)
    reg('lnr_g', [128, 1], F32)
    reg('lnr_b', [128, 1], F32)
    reg('ident', [128, 128], F16)
    reg('ones128', [128, 1], F16)
    reg('eps1', [128, 1], F32)

    brow1 = nc.dram_tensor("brow1", [16, LC], F16)
    crow1 = nc.dram_tensor("crow1", [16, LC], F16)
    brow2 = nc.dram_tensor("brow2", [16, LC], F16)
    crow2 = nc.dram_tensor("crow2", [16, LC], F16)
    lnrows = nc.dram_tensor("lnrows", [6, LC], F16)

    with tile.TileContext(nc) as tc, ExitStack() as ctx:
        sb1 = ctx.enter_context(tc.tile_pool(name="sb1", bufs=1))
        sb2 = ctx.enter_context(tc.tile_pool(name="sb2", bufs=2))
        mmp = ctx.enter_context(tc.tile_pool(name="mmp", bufs=2,
                                             space="PSUM"))
        yp = ctx.enter_context(tc.tile_pool(name="yp", bufs=5,
                                            space="PSUM"))
        vec = ctx.enter_context(tc.tile_pool(name="vec", bufs=1))
        wpool = ctx.enter_context(tc.tile_pool(name="wp", bufs=1))
        dma = _DmaRR(nc)

        Wt = {}
        for name, t in dram_w.items():
            tl = wpool.tile(list(t.shape), t.dtype, tag=name)
            dma.wload(tl[:], t.ap())
            Wt[name] = tl
        class _SbMux:
            P1 = {"xc_0", "xc_1", "sres_0", "sres_1", "xdbl", "dt32",
                  "delta", "sigout", "w16", "r_n", "h1n", "h2n", "hlast_0",
                  "hlast_1", "lnbc0", "lnbc1",
                  "lnsq", "lnt1"}

            B3 = {"bbc", "cbc"}

            def tile(self, shape, dt, tag, name=None):
                pool = sb1 if tag in self.P1 else sb2
                bufs = 3 if tag in self.B3 else None
                return pool.tile(shape, dt, tag=tag, name=name or tag,
                                 bufs=bufs)

        pools = dict(sb=_SbMux(), mm=mmp, yacc=yp, vec=vec,
                     ident=Wt['ident'], ones128=Wt['ones128'],
                     eps1=Wt['eps1'])

        xt = wpool.tile([IN_C, LPAD], F16, tag="xt")
        dma(xt[:], x_d.ap())

        # mamba 1 (includes out_proj + LN1)
        h1n = pools["sb"].tile([128, LC + 3], F16, tag="h1n")
        nc.vector.memset(h1n[:, 0:3], 0.0)
        _mamba(nc, pools, dma, Wt, 1, xt, 3, 128, 4, brow1, crow1, 'WoT',
               lnrows, 2, Wt['ln1_g'], Wt['ln1_b'], h1n, 3)

        # mamba 2
        h2n = pools["sb"].tile([128, LC], F16, tag="h2n")
        _mamba(nc, pools, dma, Wt, 2, h1n, 3, 256, 8, brow2, crow2, 'WoT',
               lnrows, 4, Wt['ln2_g'], Wt['ln2_b'], h2n, 0)

        # residual linear branch (emitted last; fills the pipeline tail)
        r_raw = pools["sb"].tile([128, LC], F16, tag="rawbuf")
        for (s, e) in _blocks(LC):
            mm = mmp.tile([128, BLK], F32, tag="mm", name="mm")
            nc.tensor.matmul(mm[:, :e - s], Wt['WlinT'][:],
                             xt[:, 3 + s: 3 + e], start=True, stop=True)
            nc.scalar.activation(r_raw[:, s:e], mm[:, :e - s], AF.Identity,
                                 bias=Wt['linb'][:])
        r_n = pools["sb"].tile([128, LC], F16, tag="r_n")
        _layer_norm(nc, pools, dma, lnrows, 0, r_raw,
                    Wt['lnr_g'], Wt['lnr_b'], r_n)

        # final: out = r + h2n (last 2048 columns)
        for (s, e) in _blocks(2048):
            fin = pools["sb"].tile([128, BLK], F32, tag="fin")
            nc.vector.tensor_tensor(out=fin[:, :e - s],
                                    in0=r_n[:, OV + s:OV + e],
                                    in1=h2n[:, OV + s:OV + e], op=OP.add)
            dma(out_d.ap()[:, s:e], fin[:, :e - s])

    nc.compile()
    return nc


_NC_CACHE = {}


def _get_nc():
    if 'nc' not in _NC_CACHE:
        _NC_CACHE['nc'] = build_nc()
    return _NC_CACHE['nc']


def _host_weights(params):
    out = {}
    for lay, key, di in ((1, 'm1', 128), (2, 'm2', 256)):
        P = params[key]
        Win = np.asarray(P['in_proj'])          # [2di, dm]
        convw = np.asarray(P['conv_w'])         # [di, 4]
        for g in range(di // 128):
            rows = slice(g * 128, (g + 1) * 128)
            for j in range(D_CONV):
                Mj = (convw[rows, j:j + 1] * Win[:di][rows]).T
                out[f'Mj{lay}_{j}_{g}'] = np.ascontiguousarray(Mj).astype(np.float16)
            out[f'Wres{lay}_{g}'] = np.ascontiguousarray(
                Win[di:][rows].T).astype(np.float16)
            out[f'convb{lay}_{g}'] = np.asarray(P['conv_b'])[rows, None].astype(np.float32)
            xp = np.asarray(P['x_proj']).copy()
            dtr = {1: 4, 2: 8}[lay]
            xp[dtr:dtr + 16] = -xp[dtr:dtr + 16]      # B rows sign-flipped
            out[f'xpT{lay}_{g}'] = np.ascontiguousarray(
                xp[:, rows].T).astype(np.float16)
            out[f'dtwT{lay}_{g}'] = np.ascontiguousarray(
                np.asarray(P['dt_w'])[rows].T).astype(np.float32)
            out[f'dtbn{lay}_{g}'] = (-np.asarray(P['dt_b']))[rows, None].astype(np.float32)
            out[f'A{lay}_{g}'] = np.ascontiguousarray(
                np.exp(np.asarray(P['A_log'])[rows])).astype(np.float32)
            out[f'D{lay}_{g}'] = np.asarray(P['D'])[rows, None].astype(np.float32)
            out[f'WoT{lay}_{g}'] = np.ascontiguousarray(
                np.asarray(P['out_proj'])[:, rows].T).astype(np.float16)
    out['ln1_g'] = np.asarray(params['ln1_g'])[:, None].astype(np.float32)
    out['ln1_b'] = np.asarray(params['ln1_b'])[:, None].astype(np.float32)
    out['ln2_g'] = np.asarray(params['ln2_g'])[:, None].astype(np.float32)
    out['ln2_b'] = np.asarray(params['ln2_b'])[:, None].astype(np.float32)
    out['WlinT'] = np.ascontiguousarray(
        np.asarray(params['lin_w']).T).astype(np.float16)
    out['linb'] = np.asarray(params['lin_b'])[:, None].astype(np.float32)
    out['lnr_g'] = np.asarray(params['ln_r_g'])[:, None].astype(np.float32)
    out['lnr_b'] = np.asarray(params['ln_r_b'])[:, None].astype(np.float32)
    out['ident'] = np.eye(128, dtype=np.float16)
    out['ones128'] = np.full((128, 1), 1.0 / 128, np.float16)
    out['eps1'] = np.full((128, 1), 1e-5, np.float32)
    return out


def kernel(x, params):
    x = np.asarray(x, np.float32)
    nc = _get_nc()
    wts = _host_weights(params)
    in_maps = []
    for c in range(8):
        b, half = c // 2, c % 2
        start = half * 2048 - OV
        xs = np.zeros((IN_C, LPAD), np.float16)
        lo = start - 3
        src_lo = max(lo, 0)
        src_hi = start + LC
        xs[:, src_lo - lo: src_lo - lo + (src_hi - src_lo)] = \
            x[b, src_lo:src_hi, :].T.astype(np.float16)
        m = dict(wts)
        m['x_t'] = xs
        in_maps.append(m)
    res = run_bass_kernel_spmd(nc, in_maps, core_ids=list(range(8)))
    _NC_CACHE['last_result'] = res
    out = np.zeros((B, L, 128), np.float32)
    for c in range(8):
        b, half = c // 2, c % 2
        out[b, half * 2048:(half + 1) * 2048, :] = res.results[c]['out'].T
    return out


# revision 21
# speedup vs baseline: 1.0345x; 1.0345x over previous
"""Trainium2 Bass kernel for the double-Mamba block (nn_DoubleConv).

Sharding: 8 cores = 4 batches x 2 sequence halves. Each core processes
LC = 2048 + OV columns of its batch element; OV columns are burn-in
(delta >= 0.54 => per-step state decay <= e^-0.54, so OV/2 columns per
mamba layer push the truncation error below fp32 noise).

Layout: features on partitions, time on the free axis.
 - causal depthwise conv folded into in_proj: 4 accumulating PE matmuls
   with shifted rhs, lhsT_j = (conv_w[:, j] * W_in_xc).T
 - dA_n = Exp(delta * A[:, n]) on ScalarE (per-partition scale AP)
 - h_n via VectorE tensor_tensor_scan (fp32 state)
 - B/C rows broadcast across partitions via DRAM round-trip DMAs
 - y = sum_n C_n*h_n via PE identity-matmul PSUM accumulation
 - LayerNorm over the feature(partition) axis via ones/128 matmuls
"""
import numpy as np
from contextlib import ExitStack

import concourse.bass as bass
import concourse.bacc as bacc
import concourse.mybir as mybir
import concourse.tile as tile
from concourse.bass_utils import run_bass_kernel_spmd

F32 = mybir.dt.float32
F16 = mybir.dt.float16
AF = mybir.ActivationFunctionType
OP = mybir.AluOpType

D_STATE = 16
D_CONV = 4
B, L, IN_C, OUT_C = 4, 4096, 64, 128
OV = 128                      # burn-in columns (covers both layers)
LC = 2048 + OV                # per-core columns
LPAD = LC + 3                 # conv left-pad
BLK = 512                     # PSUM block


def _blocks(n, bs=BLK):
    return [(s, min(s + bs, n)) for s in range(0, n, bs)]


class _DmaRR:
    """DMA issue router: bulk broadcasts alternate Sync/GpSimd queues so
    the Scalar sequencer stays free for ACTIVATE dispatch."""

    def __init__(self, nc):
        self.nc = nc
        self.i = 0

    def __call__(self, out, in_):
        e = [self.nc.sync, self.nc.gpsimd][self.i % 2]
        self.i += 1
        return e.dma_start(out, in_)

    def wload(self, out, in_):
        return self.nc.scalar.dma_start(out, in_)


def _layer_norm(nc, pools, dma, lnrows_dram, row_base,
                h_raw, g_col, b_col, out_tile, out_off=0, col0=0, ncols=LC):
    """LN over the partition axis of h_raw[:, col0:col0+ncols] (f16, SBUF).
    Writes f16 into out_tile[:, out_off+col0 : out_off+col0+ncols]."""
    sb, mmp, vec = pools['sb'], pools['mm'], pools['vec']
    ones_over = pools['ones128']     # [128, 1] f16 of 1/128
    c1 = col0 + ncols
    h_sq = sb.tile([128, LC], F16, tag="lnsq", name="lnsq")
    nc.scalar.activation(h_sq[:, col0:c1], h_raw[:, col0:c1], AF.Square)
    vA = vec.tile([128, LC], F32, tag="vA", name="vA")
    vB = vec.tile([128, LC], F32, tag="vB", name="vB")
    msq, var, mu = vA[0:1, :], vA[32:33, :], vA[64:65, :]
    mu2, s_row = vB[0:1, :], vB[64:65, :]
    mus_row = vA[96:97, :]
    for (s, e) in _blocks(ncols):
        s, e = s + col0, e + col0
        p1 = mmp.tile([1, BLK], F32, tag="mm", name="mm")
        nc.tensor.matmul(p1[:, :e - s], ones_over[:], h_raw[:, s:e],
                         start=True, stop=True)
        nc.scalar.activation(mu[:, s:e], p1[:, :e - s], AF.Copy)
        p2 = mmp.tile([1, BLK], F32, tag="mm", name="mm")
        nc.tensor.matmul(p2[:, :e - s], ones_over[:], h_sq[:, s:e],
                         start=True, stop=True)
        nc.scalar.activation(msq[:, s:e], p2[:, :e - s], AF.Copy)
    nc.scalar.activation(mu2[:, col0:c1], mu[:, col0:c1], AF.Square)
    nc.vector.tensor_tensor(out=var[:, col0:c1], in0=msq[:, col0:c1],
                            in1=mu2[:, col0:c1], op=OP.subtract)
    nc.scalar.activation(var[:, col0:c1], var[:, col0:c1], AF.Ln,
                         bias=pools['eps1'][:1, :])
    nc.scalar.activation(s_row[:, col0:c1], var[:, col0:c1], AF.Exp,
                         scale=-0.5)
    nc.vector.tensor_tensor(out=mus_row[:, col0:c1], in0=mu[:, col0:c1],
                            in1=s_row[:, col0:c1], op=OP.mult)
    # f32 -> f16 cast happens inside the gpsimd software-DGE DMA
    nc.gpsimd.dma_start(out=lnrows_dram.ap()[row_base:row_base + 1, col0:c1],
                        in_=s_row[:, col0:c1])
    nc.gpsimd.dma_start(
        out=lnrows_dram.ap()[row_base + 1:row_base + 2, col0:c1],
        in_=mus_row[:, col0:c1])
    s_bc = sb.tile([128, LC], F16, tag="lnbc0", name="lnbc0")
    mus_bc = sb.tile([128, LC], F16, tag="lnbc1", name="lnbc1")
    dma(s_bc[:, col0:c1], lnrows_dram.ap()[row_base:row_base + 1, col0:c1]
        .broadcast_to((128, ncols)))
    dma(mus_bc[:, col0:c1],
        lnrows_dram.ap()[row_base + 1:row_base + 2, col0:c1]
        .broadcast_to((128, ncols)))
    # out = ((h*s_bc) - mus_bc)*g + b
    t1 = sb.tile([128, LC], F16, tag="lnt1", name="lnt1")
    nc.vector.tensor_tensor(out=t1[:, col0:c1], in0=h_raw[:, col0:c1],
                            in1=s_bc[:, col0:c1], op=OP.mult)
    t2 = sb.tile([128, LC], F16, tag="lnsq", name="lnsq")
    nc.vector.tensor_tensor(out=t2[:, col0:c1], in0=t1[:, col0:c1],
                            in1=mus_bc[:, col0:c1], op=OP.subtract)
    nc.vector.tensor_scalar(out=out_tile[:, out_off + col0:out_off + c1],
                            in0=t2[:, col0:c1], scalar1=g_col[:],
                            scalar2=b_col[:], op0=OP.mult, op1=OP.add)


def _mamba(nc, pools, dma, W, lay, xin, xin_off, di, dtr,
           brow_dram, crow_dram, n_wo_grp, lnrows_dram, ln_row, g_col,
           b_col, out_norm, out_norm_off):
    """One mamba layer, processed in 2 time-chunks so chunk-1 prep overlaps
    chunk-0 scans. Includes out_proj and LayerNorm. Writes normalized f16
    into out_norm[:, out_norm_off : out_norm_off+LC]."""
    sb, mmp, yp = pools['sb'], pools['mm'], pools['yacc']
    n_grp = di // 128
    HC = LC // 2
    hlast = [sb.tile([128, 16], F16, tag=f"hlast_{g}", name="hlast")
             for g in range(n_grp)]
    o_raw = sb.tile([128, LC], F16, tag="rawbuf", name="rawbuf")
    for c in range(2):
        c0 = c * HC
        cols = [(c0 + s_, c0 + e_) for (s_, e_) in _blocks(HC)]
        xc2 = [sb.tile([128, LC], F16, tag=f"xc_{g}", name="xc")
               for g in range(n_grp)] if c == 0 else _mamba.xc2
        sres = [sb.tile([128, LC], F16, tag=f"sres_{g}", name="sres")
                for g in range(n_grp)] if c == 0 else _mamba.sres
        if c == 0:
            _mamba.xc2, _mamba.sres = xc2, sres
        for g in range(n_grp):
            for (s, e) in cols:
                mm = mmp.tile([128, BLK], F32, tag="mm", name="mm")
                for j in range(D_CONV):
                    nc.tensor.matmul(
                        mm[:, :e - s], W[f'Mj{lay}_{j}_{g}'][:],
                        xin[:, xin_off - 3 + j + s: xin_off - 3 + j + e],
                        start=(j == 0), stop=(j == D_CONV - 1))
                nc.scalar.activation(xc2[g][:, s:e], mm[:, :e - s], AF.Silu,
                                     bias=W[f'convb{lay}_{g}'][:])
                mm2 = mmp.tile([128, BLK], F32, tag="mm", name="mm")
                nc.tensor.matmul(mm2[:, :e - s], W[f'Wres{lay}_{g}'][:],
                                 xin[:, xin_off + s: xin_off + e],
                                 start=True, stop=True)
                nc.scalar.activation(sres[g][:, s:e], mm2[:, :e - s], AF.Silu)
        nxd = dtr + 32
        xdbl16 = sb.tile([nxd, LC], F16, tag="xdbl", name="xdbl") \
            if c == 0 else _mamba.xdbl
        dt32 = sb.tile([dtr, LC], F32, tag="dt32", name="dt32") \
            if c == 0 else _mamba.dt32
        if c == 0:
            _mamba.xdbl, _mamba.dt32 = xdbl16, dt32
        for (s, e) in cols:
            mm = mmp.tile([nxd, BLK], F32, tag="mm", name="mm")
            for g in range(n_grp):
                nc.tensor.matmul(mm[:, :e - s], W[f'xpT{lay}_{g}'][:],
                                 xc2[g][:, s:e],
                                 start=(g == 0), stop=(g == n_grp - 1))
            nc.scalar.activation(xdbl16[:, s:e], mm[:, :e - s], AF.Copy)
            nc.scalar.activation(dt32[:, s:e], mm[:dtr, :e - s], AF.Copy)
        dma(brow_dram.ap()[:, c0:c0 + HC], xdbl16[dtr:dtr + 16, c0:c0 + HC])
        dma(crow_dram.ap()[:, c0:c0 + HC],
            xdbl16[dtr + 16:dtr + 32, c0:c0 + HC])
        for g in range(n_grp):
            # delta = softplus(pre + dt_b) = ln(1 + exp(pre + dt_b))
            delta = sb.tile([128, HC], F32, tag="delta", name="delta")
            sigout = sb.tile([128, HC], F32, tag="sigout", name="sigout")
            for (s, e) in cols:
                mm = mmp.tile([128, BLK], F32, tag="mm", name="mm")
                nc.tensor.matmul(mm[:, :e - s], W[f'dtwT{lay}_{g}'][:],
                                 dt32[:, s:e], start=True, stop=True)
                nc.scalar.activation(sigout[:, s - c0:e - c0], mm[:, :e - s],
                                     AF.Exp, bias=W[f'dtb{lay}_{g}'][:])
            nc.scalar.activation(delta[:], sigout[:], AF.Ln, bias=1.0)
            w16 = sb.tile([128, HC], F16, tag="w16", name="w16")
            nc.vector.tensor_tensor(out=w16[:], in0=delta[:],
                                    in1=xc2[g][:, c0:c0 + HC], op=OP.mult)
            ytiles = [yp.tile([128, BLK], F32, tag="yacc", name="yacc")
                      for _ in cols]

            def _emit_q(n, h, c_bc):
                q = sb.tile([128, HC], F16, tag="q", name="q")
                nc.vector.tensor_tensor(out=q[:], in0=h[:], in1=c_bc[:],
                                        op=OP.mult)
                for bi, (s, e) in enumerate(cols):
                    nc.tensor.matmul(ytiles[bi][:, :e - s], pools['ident'][:],
                                     q[:, s - c0:e - c0],
                                     start=(n == 0), stop=(n == 15))

            prev = None
            for n in range(16):
                dA = sb.tile([128, HC], F32, tag="dA", name="dA")
                nc.scalar.activation(dA[:], delta[:], AF.Exp,
                                     scale=W[f'A{lay}_{g}'][:, n:n + 1])
                b_bc = sb.tile([128, HC], F16, tag="bbc", name="bbc")
                dma(b_bc[:], brow_dram.ap()[n:n + 1, c0:c0 + HC]
                    .broadcast_to((128, HC)))
                c_bc = sb.tile([128, HC], F16, tag="cbc", name="cbc")
                dma(c_bc[:], crow_dram.ap()[n:n + 1, c0:c0 + HC]
                    .broadcast_to((128, HC)))
                dBu = sb.tile([128, HC], F16, tag="dbu", name="dbu")
                nc.vector.tensor_tensor(out=dBu[:], in0=w16[:], in1=b_bc[:],
                                        op=OP.mult)
                if prev is not None:
                    _emit_q(*prev)
                h = sb.tile([128, HC], F16, tag="h", name="h")
                init = 0.0 if c == 0 else hlast[g][:, n:n + 1]
                nc.vector.tensor_tensor_scan(h[:], dA[:], dBu[:], init,
                                             OP.mult, OP.add)
                if c == 0:
                    nc.vector.tensor_copy(hlast[g][:, n:n + 1], h[:, -1:])
                prev = (n, h, c_bc)
            _emit_q(*prev)
            m_raw = sb.tile([128, HC], F16, tag=f"mraw_{g}", name="mraw") \
                if True else None
            for bi, (s, e) in enumerate(cols):
                t1 = sb.tile([128, BLK], F32, tag="gt1", name="gt1")
                nc.vector.scalar_tensor_tensor(
                    t1[:, :e - s], xc2[g][:, s:e], W[f'D{lay}_{g}'][:],
                    ytiles[bi][:, :e - s], OP.mult, OP.add)
                nc.vector.tensor_tensor(out=m_raw[:, s - c0:e - c0],
                                        in0=t1[:, :e - s],
                                        in1=sres[g][:, s:e], op=OP.mult)
            if g == 0:
                _mamba.mraws = []
            _mamba.mraws.append(m_raw)
        # out_proj over groups, then LN on this chunk
        for (s, e) in cols:
            mm = mmp.tile([128, BLK], F32, tag="mm", name="mm")
            for g in range(n_grp):
                nc.tensor.matmul(mm[:, :e - s], W[f'{n_wo_grp}{lay}_{g}'][:],
                                 _mamba.mraws[g][:, s - c0:e - c0],
                                 start=(g == 0), stop=(g == n_grp - 1))
            nc.scalar.activation(o_raw[:, s:e], mm[:, :e - s], AF.Copy)
        _layer_norm(nc, pools, dma, lnrows_dram, ln_row,
                    o_raw, g_col, b_col, out_norm,
                    out_off=out_norm_off, col0=c0, ncols=HC)


def build_nc():
    nc = bacc.Bacc("TRN2", target_bir_lowering=False, debug=False)
    dram_w = {}

    def reg(name, shape, dt):
        dram_w[name] = nc.dram_tensor(name, list(shape), dt,
                                      kind="ExternalInput")

    x_d = nc.dram_tensor("x_t", [IN_C, LPAD], F16, kind="ExternalInput")
    out_d = nc.dram_tensor("out", [128, 2048], F32, kind="ExternalOutput")
    reg('w64', [64, 6 * 128], F16)
    reg('w128', [128, 14 * 128], F16)
    reg('wxp', [128, 116], F16)
    reg('wdt', [8, 384], F32)
    reg('wA', [128, 48], F32)
    reg('wcols', [128, 17], F32)
    reg('ones128', [128, 1], F16)

    brow1 = nc.dram_tensor("brow1", [16, LC], F16)
    crow1 = nc.dram_tensor("crow1", [16, LC], F16)
    brow2 = nc.dram_tensor("brow2", [16, LC], F16)
    crow2 = nc.dram_tensor("crow2", [16, LC], F16)
    lnrows = nc.dram_tensor("lnrows", [6, LC], F16)

    with tile.TileContext(nc) as tc, ExitStack() as ctx:
        sb1 = ctx.enter_context(tc.tile_pool(name="sb1", bufs=1))
        sb2 = ctx.enter_context(tc.tile_pool(name="sb2", bufs=2))
        mmp = ctx.enter_context(tc.tile_pool(name="mmp", bufs=2,
                                             space="PSUM"))
        yp = ctx.enter_context(tc.tile_pool(name="yp", bufs=5,
                                            space="PSUM"))
        vec = ctx.enter_context(tc.tile_pool(name="vec", bufs=1))
        wpool = ctx.enter_context(tc.tile_pool(name="wp", bufs=1))
        dma = _DmaRR(nc)

        # x first; packed weights in a handful of DMAs on the scalar queue
        xt = wpool.tile([IN_C, LPAD], F16, tag="xt")
        nc.sync.dma_start(xt[:], x_d.ap())
        packed = {}
        for name in ['w64', 'wdt', 'wxp', 'wcols', 'wA', 'w128', 'ones128']:
            t = dram_w[name]
            tl = wpool.tile(list(t.shape), t.dtype, tag=name, name=name)
            dma.wload(tl[:], t.ap())
            packed[name] = tl
        w64, w128 = packed['w64'], packed['w128']
        Wt = {'ones128': packed['ones128']}
        for k, nm in enumerate(['Mj1_0_0', 'Mj1_1_0', 'Mj1_2_0', 'Mj1_3_0',
                                'Wres1_0', 'WlinT']):
            Wt[nm] = w64[:, k * 128:(k + 1) * 128]
        for k, nm in enumerate(['Mj2_0_0', 'Mj2_1_0', 'Mj2_2_0', 'Mj2_3_0',
                                'Mj2_0_1', 'Mj2_1_1', 'Mj2_2_1', 'Mj2_3_1',
                                'Wres2_0', 'Wres2_1', 'WoT1_0', 'WoT2_0',
                                'WoT2_1', 'ident']):
            Wt[nm] = w128[:, k * 128:(k + 1) * 128]
        Wt['xpT1_0'] = packed['wxp'][:, 0:36]
        Wt['xpT2_0'] = packed['wxp'][:, 36:76]
        Wt['xpT2_1'] = packed['wxp'][:, 76:116]
        Wt['dtwT1_0'] = packed['wdt'][0:4, 0:128]
        Wt['dtwT2_0'] = packed['wdt'][:, 128:256]
        Wt['dtwT2_1'] = packed['wdt'][:, 256:384]
        Wt['A1_0'] = packed['wA'][:, 0:16]
        Wt['A2_0'] = packed['wA'][:, 16:32]
        Wt['A2_1'] = packed['wA'][:, 32:48]
        for k, nm in enumerate(['convb1_0', 'dtb1_0', 'D1_0', 'convb2_0',
                                'convb2_1', 'dtb2_0', 'dtb2_1', 'D2_0',
                                'D2_1', 'ln1_g', 'ln1_b', 'ln2_g', 'ln2_b',
                                'lnr_g', 'lnr_b', 'linb', 'eps1']):
            Wt[nm] = packed['wcols'][:, k:k + 1]
        class _SbMux:
            P1 = {"xc_0", "xc_1", "sres_0", "sres_1", "xdbl", "dt32",
                  "delta", "sigout", "w16", "r_n", "h1n", "h2n", "hlast_0",
                  "hlast_1", "lnbc0", "lnbc1",
                  "lnsq", "lnt1"}

            B3 = {"bbc", "cbc"}

            def tile(self, shape, dt, tag, name=None):
                pool = sb1 if tag in self.P1 else sb2
                bufs = 3 if tag in self.B3 else None
                return pool.tile(shape, dt, tag=tag, name=name or tag,
                                 bufs=bufs)

        pools = dict(sb=_SbMux(), mm=mmp, yacc=yp, vec=vec,
                     ident=Wt['ident'], ones128=Wt['ones128'],
                     eps1=Wt['eps1'])

        # mamba 1 (includes out_proj + LN1)
        h1n = pools["sb"].tile([128, LC + 3], F16, tag="h1n")
        nc.vector.memset(h1n[:, 0:3], 0.0)
        _mamba(nc, pools, dma, Wt, 1, xt, 3, 128, 4, brow1, crow1, 'WoT',
               lnrows, 2, Wt['ln1_g'], Wt['ln1_b'], h1n, 3)

        # mamba 2
        h2n = pools["sb"].tile([128, LC], F16, tag="h2n")
        _mamba(nc, pools, dma, Wt, 2, h1n, 3, 256, 8, brow2, crow2, 'WoT',
               lnrows, 4, Wt['ln2_g'], Wt['ln2_b'], h2n, 0)

        # residual linear branch (emitted last; fills the pipeline tail)
        r_raw = pools["sb"].tile([128, LC], F16, tag="rawbuf")
        for (s, e) in _blocks(LC):
            mm = mmp.tile([128, BLK], F32, tag="mm", name="mm")
            nc.tensor.matmul(mm[:, :e - s], Wt['WlinT'][:],
                             xt[:, 3 + s: 3 + e], start=True, stop=True)
            nc.scalar.activation(r_raw[:, s:e], mm[:, :e - s], AF.Identity,
                                 bias=Wt['linb'][:])
        r_n = pools["sb"].tile([128, LC], F16, tag="r_n")
        _layer_norm(nc, pools, dma, lnrows, 0, r_raw,
                    Wt['lnr_g'], Wt['lnr_b'], r_n)

        # final: out = r + h2n (last 2048 columns)
        for (s, e) in _blocks(2048):
            fin = pools["sb"].tile([128, BLK], F32, tag="fin")
            nc.vector.tensor_tensor(out=fin[:, :e - s],
                                    in0=r_n[:, OV + s:OV + e],
                                    in1=h2n[:, OV + s:OV + e], op=OP.add)
            dma(out_d.ap()[:, s:e], fin[:, :e - s])

    nc.compile()
    return nc


_NC_CACHE = {}


def _get_nc():
    if 'nc' not in _NC_CACHE:
        _NC_CACHE['nc'] = build_nc()
    return _NC_CACHE['nc']


def _host_weights(params):
    out = {}
    for lay, key, di in ((1, 'm1', 128), (2, 'm2', 256)):
        P = params[key]
        Win = np.asarray(P['in_proj'])          # [2di, dm]
        convw = np.asarray(P['conv_w'])         # [di, 4]
        for g in range(di // 128):
            rows = slice(g * 128, (g + 1) * 128)
            for j in range(D_CONV):
                Mj = (convw[rows, j:j + 1] * Win[:di][rows]).T
                out[f'Mj{lay}_{j}_{g}'] = np.ascontiguousarray(Mj).astype(np.float16)
            out[f'Wres{lay}_{g}'] = np.ascontiguousarray(
                Win[di:][rows].T).astype(np.float16)
            out[f'convb{lay}_{g}'] = np.asarray(P['conv_b'])[rows, None].astype(np.float32)
            out[f'xpT{lay}_{g}'] = np.ascontiguousarray(
                np.asarray(P['x_proj'])[:, rows].T).astype(np.float16)
            out[f'dtwT{lay}_{g}'] = np.ascontiguousarray(
                np.asarray(P['dt_w'])[rows].T).astype(np.float32)
            out[f'dtb{lay}_{g}'] = np.asarray(P['dt_b'])[rows, None].astype(np.float32)
            out[f'A{lay}_{g}'] = np.ascontiguousarray(
                -np.exp(np.asarray(P['A_log'])[rows])).astype(np.float32)
            out[f'D{lay}_{g}'] = np.asarray(P['D'])[rows, None].astype(np.float32)
            out[f'WoT{lay}_{g}'] = np.ascontiguousarray(
                np.asarray(P['out_proj'])[:, rows].T).astype(np.float16)
    out['ln1_g'] = np.asarray(params['ln1_g'])[:, None].astype(np.float32)
    out['ln1_b'] = np.asarray(params['ln1_b'])[:, None].astype(np.float32)
    out['ln2_g'] = np.asarray(params['ln2_g'])[:, None].astype(np.float32)
    out['ln2_b'] = np.asarray(params['ln2_b'])[:, None].astype(np.float32)
    out['WlinT'] = np.ascontiguousarray(
        np.asarray(params['lin_w']).T).astype(np.float16)
    out['linb'] = np.asarray(params['lin_b'])[:, None].astype(np.float32)
    out['lnr_g'] = np.asarray(params['ln_r_g'])[:, None].astype(np.float32)
    out['lnr_b'] = np.asarray(params['ln_r_b'])[:, None].astype(np.float32)
    out['ident'] = np.eye(128, dtype=np.float16)
    out['eps1'] = np.full((128, 1), 1e-5, np.float32)

    # pack into the device's fused input tensors
    packed = {}
    packed['w64'] = np.concatenate(
        [out[n] for n in ['Mj1_0_0', 'Mj1_1_0', 'Mj1_2_0', 'Mj1_3_0',
                          'Wres1_0', 'WlinT']], axis=1)
    packed['w128'] = np.concatenate(
        [out[n] for n in ['Mj2_0_0', 'Mj2_1_0', 'Mj2_2_0', 'Mj2_3_0',
                          'Mj2_0_1', 'Mj2_1_1', 'Mj2_2_1', 'Mj2_3_1',
                          'Wres2_0', 'Wres2_1', 'WoT1_0', 'WoT2_0',
                          'WoT2_1', 'ident']], axis=1)
    packed['wxp'] = np.concatenate(
        [out['xpT1_0'], out['xpT2_0'], out['xpT2_1']], axis=1)
    wdt = np.zeros((8, 384), np.float32)
    wdt[0:4, 0:128] = out['dtwT1_0']
    wdt[:, 128:256] = out['dtwT2_0']
    wdt[:, 256:384] = out['dtwT2_1']
    packed['wdt'] = wdt
    packed['wA'] = np.concatenate(
        [out['A1_0'], out['A2_0'], out['A2_1']], axis=1)
    packed['wcols'] = np.concatenate(
        [out[n] for n in ['convb1_0', 'dtb1_0', 'D1_0', 'convb2_0',
                          'convb2_1', 'dtb2_0', 'dtb2_1', 'D2_0', 'D2_1',
                          'ln1_g', 'ln1_b', 'ln2_g', 'ln2_b', 'lnr_g',
                          'lnr_b', 'linb', 'eps1']], axis=1)
    packed['ones128'] = np.full((128, 1), 1.0 / 128, np.float16)
    return packed


def kernel(x, params):
    x = np.asarray(x, np.float32)
    nc = _get_nc()
    wts = _host_weights(params)
    in_maps = []
    for c in range(8):
        b, half = c // 2, c % 2
        start = half * 2048 - OV
        xs = np.zeros((IN_C, LPAD), np.float16)
        lo = start - 3
        src_lo = max(lo, 0)
        src_hi = start + LC
        xs[:, src_lo - lo: src_lo - lo + (src_hi - src_lo)] = \
            x[b, src_lo:src_hi, :].T.astype(np.float16)
        m = dict(wts)
        m['x_t'] = xs
        in_maps.append(m)
    res = run_bass_kernel_spmd(nc, in_maps, core_ids=list(range(8)))
    _NC_CACHE['last_result'] = res
    out = np.zeros((B, L, 128), np.float32)
    for c in range(8):
        b, half = c // 2, c % 2
        out[b, half * 2048:(half + 1) * 2048, :] = res.results[c]['out'].T
    return out


# revision 22
# speedup vs baseline: 1.0459x; 1.0110x over previous
"""Trainium2 Bass kernel for the double-Mamba block (nn_DoubleConv).

Sharding: 8 cores = 4 batches x 2 sequence halves. Each core processes
LC = 2048 + OV columns of its batch element; OV columns are burn-in
(delta >= 0.54 => per-step state decay <= e^-0.54, so OV/2 columns per
mamba layer push the truncation error below fp32 noise).

Layout: features on partitions, time on the free axis.
 - causal depthwise conv folded into in_proj: 4 accumulating PE matmuls
   with shifted rhs, lhsT_j = (conv_w[:, j] * W_in_xc).T
 - dA_n = Exp(delta * A[:, n]) on ScalarE (per-partition scale AP)
 - h_n via VectorE tensor_tensor_scan (fp32 state)
 - B/C rows broadcast across partitions via DRAM round-trip DMAs
 - y = sum_n C_n*h_n via PE identity-matmul PSUM accumulation
 - LayerNorm over the feature(partition) axis via ones/128 matmuls
"""
import numpy as np
from contextlib import ExitStack

import concourse.bass as bass
import concourse.bacc as bacc
import concourse.mybir as mybir
import concourse.tile as tile
from concourse.bass_utils import run_bass_kernel_spmd

F32 = mybir.dt.float32
F16 = mybir.dt.float16
AF = mybir.ActivationFunctionType
OP = mybir.AluOpType

D_STATE = 16
D_CONV = 4
B, L, IN_C, OUT_C = 4, 4096, 64, 128
OV = 128                      # burn-in columns (covers both layers)
LC = 2048 + OV                # per-core columns
LPAD = LC + 3                 # conv left-pad
BLK = 512                     # PSUM block


def _blocks(n, bs=BLK):
    return [(s, min(s + bs, n)) for s in range(0, n, bs)]


class _DmaRR:
    """DMA issue router: bulk broadcasts alternate Sync/GpSimd queues so
    the Scalar sequencer stays free for ACTIVATE dispatch."""

    def __init__(self, nc):
        self.nc = nc
        self.i = 0

    def __call__(self, out, in_):
        e = [self.nc.sync, self.nc.gpsimd][self.i % 2]
        self.i += 1
        return e.dma_start(out, in_)

    def wload(self, out, in_):
        return self.nc.scalar.dma_start(out, in_)


def _layer_norm(nc, pools, dma, lnrows_dram, row_base,
                h_raw, g_col, b_col, out_tile, out_off=0, col0=0, ncols=LC):
    """LN over the partition axis of h_raw[:, col0:col0+ncols] (f16, SBUF).
    Writes f16 into out_tile[:, out_off+col0 : out_off+col0+ncols]."""
    sb, mmp, vec = pools['sb'], pools['mm'], pools['vec']
    ones_over = pools['ones128']     # [128, 1] f16 of 1/128
    c1 = col0 + ncols
    h_sq = sb.tile([128, LC], F16, tag="lnsq", name="lnsq")
    nc.scalar.activation(h_sq[:, col0:c1], h_raw[:, col0:c1], AF.Square)
    vA = vec.tile([128, LC], F32, tag="vA", name="vA")
    vB = vec.tile([128, LC], F32, tag="vB", name="vB")
    msq, var, mu = vA[0:1, :], vA[32:33, :], vA[64:65, :]
    mu2, s_row = vB[0:1, :], vB[64:65, :]
    mus_row = vA[96:97, :]
    for (s, e) in _blocks(ncols):
        s, e = s + col0, e + col0
        p1 = mmp.tile([1, BLK], F32, tag="mm", name="mm")
        nc.tensor.matmul(p1[:, :e - s], ones_over[:], h_raw[:, s:e],
                         start=True, stop=True)
        nc.scalar.activation(mu[:, s:e], p1[:, :e - s], AF.Copy)
        p2 = mmp.tile([1, BLK], F32, tag="mm", name="mm")
        nc.tensor.matmul(p2[:, :e - s], ones_over[:], h_sq[:, s:e],
                         start=True, stop=True)
        nc.scalar.activation(msq[:, s:e], p2[:, :e - s], AF.Copy)
    nc.scalar.activation(mu2[:, col0:c1], mu[:, col0:c1], AF.Square)
    nc.vector.tensor_tensor(out=var[:, col0:c1], in0=msq[:, col0:c1],
                            in1=mu2[:, col0:c1], op=OP.subtract)
    nc.scalar.activation(var[:, col0:c1], var[:, col0:c1], AF.Ln,
                         bias=pools['eps1'][:1, :])
    nc.scalar.activation(s_row[:, col0:c1], var[:, col0:c1], AF.Exp,
                         scale=-0.5)
    nc.vector.tensor_tensor(out=mus_row[:, col0:c1], in0=mu[:, col0:c1],
                            in1=s_row[:, col0:c1], op=OP.mult)
    # f32 -> f16 cast happens inside the gpsimd software-DGE DMA
    nc.gpsimd.dma_start(out=lnrows_dram.ap()[row_base:row_base + 1, col0:c1],
                        in_=s_row[:, col0:c1])
    nc.gpsimd.dma_start(
        out=lnrows_dram.ap()[row_base + 1:row_base + 2, col0:c1],
        in_=mus_row[:, col0:c1])
    s_bc = sb.tile([128, LC], F16, tag="lnbc0", name="lnbc0")
    mus_bc = sb.tile([128, LC], F16, tag="lnbc1", name="lnbc1")
    dma(s_bc[:, col0:c1], lnrows_dram.ap()[row_base:row_base + 1, col0:c1]
        .broadcast_to((128, ncols)))
    dma(mus_bc[:, col0:c1],
        lnrows_dram.ap()[row_base + 1:row_base + 2, col0:c1]
        .broadcast_to((128, ncols)))
    # out = ((h*s_bc) - mus_bc)*g + b
    t1 = sb.tile([128, LC], F16, tag="lnt1", name="lnt1")
    nc.vector.tensor_tensor(out=t1[:, col0:c1], in0=h_raw[:, col0:c1],
                            in1=s_bc[:, col0:c1], op=OP.mult)
    t2 = sb.tile([128, LC], F16, tag="lnsq", name="lnsq")
    nc.vector.tensor_tensor(out=t2[:, col0:c1], in0=t1[:, col0:c1],
                            in1=mus_bc[:, col0:c1], op=OP.subtract)
    nc.vector.tensor_scalar(out=out_tile[:, out_off + col0:out_off + c1],
                            in0=t2[:, col0:c1], scalar1=g_col[:],
                            scalar2=b_col[:], op0=OP.mult, op1=OP.add)


def _mamba(nc, pools, dma, W, lay, xin, xin_off, di, dtr,
           brow_dram, crow_dram, n_wo_grp, lnrows_dram, ln_row, g_col,
           b_col, out_norm, out_norm_off):
    """One mamba layer, processed in 2 time-chunks so chunk-1 prep overlaps
    chunk-0 scans. Includes out_proj and LayerNorm. Writes normalized f16
    into out_norm[:, out_norm_off : out_norm_off+LC]."""
    sb, mmp, yp = pools['sb'], pools['mm'], pools['yacc']
    n_grp = di // 128
    HC = LC // 2
    hlast = [sb.tile([128, 16], F16, tag=f"hlast_{g}", name="hlast")
             for g in range(n_grp)]
    o_raw = sb.tile([128, LC], F16, tag="rawbuf", name="rawbuf")
    for c in range(2):
        c0 = c * HC
        cols = [(c0 + s_, c0 + e_) for (s_, e_) in _blocks(HC)]
        xc2 = [sb.tile([128, LC], F16, tag=f"xc_{g}", name="xc")
               for g in range(n_grp)] if c == 0 else _mamba.xc2
        sres = [sb.tile([128, LC], F16, tag=f"sres_{g}", name="sres")
                for g in range(n_grp)] if c == 0 else _mamba.sres
        if c == 0:
            _mamba.xc2, _mamba.sres = xc2, sres
        for g in range(n_grp):
            for (s, e) in cols:
                mm = mmp.tile([128, BLK], F32, tag="mm", name="mm")
                for j in range(D_CONV):
                    nc.tensor.matmul(
                        mm[:, :e - s], W[f'Mj{lay}_{j}_{g}'][:],
                        xin[:, xin_off - 3 + j + s: xin_off - 3 + j + e],
                        start=(j == 0), stop=(j == D_CONV - 1))
                nc.scalar.activation(xc2[g][:, s:e], mm[:, :e - s], AF.Silu,
                                     bias=W[f'convb{lay}_{g}'][:])
                mm2 = mmp.tile([128, BLK], F32, tag="mm", name="mm")
                nc.tensor.matmul(mm2[:, :e - s], W[f'Wres{lay}_{g}'][:],
                                 xin[:, xin_off + s: xin_off + e],
                                 start=True, stop=True)
                nc.scalar.activation(sres[g][:, s:e], mm2[:, :e - s], AF.Silu)
        nxd = dtr + 32
        xdbl16 = sb.tile([nxd, LC], F16, tag="xdbl", name="xdbl") \
            if c == 0 else _mamba.xdbl
        dt32 = sb.tile([dtr, LC], F32, tag="dt32", name="dt32") \
            if c == 0 else _mamba.dt32
        if c == 0:
            _mamba.xdbl, _mamba.dt32 = xdbl16, dt32
        for (s, e) in cols:
            mm = mmp.tile([nxd, BLK], F32, tag="mm", name="mm")
            for g in range(n_grp):
                nc.tensor.matmul(mm[:, :e - s], W[f'xpT{lay}_{g}'][:],
                                 xc2[g][:, s:e],
                                 start=(g == 0), stop=(g == n_grp - 1))
            nc.scalar.activation(xdbl16[:, s:e], mm[:, :e - s], AF.Copy)
            nc.scalar.activation(dt32[:, s:e], mm[:dtr, :e - s], AF.Copy)
        dma(brow_dram.ap()[:, c0:c0 + HC], xdbl16[dtr:dtr + 16, c0:c0 + HC])
        dma(crow_dram.ap()[:, c0:c0 + HC],
            xdbl16[dtr + 16:dtr + 32, c0:c0 + HC])
        for g in range(n_grp):
            # delta = softplus(pre + dt_b) = ln(1 + exp(pre + dt_b))
            delta = sb.tile([128, HC], F32, tag="delta", name="delta")
            sigout = sb.tile([128, HC], F32, tag="sigout", name="sigout")
            for (s, e) in cols:
                mm = mmp.tile([128, BLK], F32, tag="mm", name="mm")
                nc.tensor.matmul(mm[:, :e - s], W[f'dtwT{lay}_{g}'][:],
                                 dt32[:, s:e], start=True, stop=True)
                nc.scalar.activation(sigout[:, s - c0:e - c0], mm[:, :e - s],
                                     AF.Exp, bias=W[f'dtb{lay}_{g}'][:])
            nc.scalar.activation(delta[:], sigout[:], AF.Ln, bias=1.0)
            w16 = sb.tile([128, HC], F16, tag="w16", name="w16")
            nc.vector.tensor_tensor(out=w16[:], in0=delta[:],
                                    in1=xc2[g][:, c0:c0 + HC], op=OP.mult)
            ytiles = [yp.tile([128, BLK], F32, tag="yacc", name="yacc")
                      for _ in cols]

            def _emit_q(n, h, c_bc):
                q = sb.tile([128, HC], F16, tag="q", name="q")
                nc.vector.tensor_tensor(out=q[:], in0=h[:], in1=c_bc[:],
                                        op=OP.mult)
                for bi, (s, e) in enumerate(cols):
                    nc.tensor.matmul(ytiles[bi][:, :e - s], pools['ident'][:],
                                     q[:, s - c0:e - c0],
                                     start=(n == 0), stop=(n == 15))

            prev = None
            for n in range(16):
                dA = sb.tile([128, HC], F32, tag="dA", name="dA")
                nc.scalar.activation(dA[:], delta[:], AF.Exp,
                                     scale=W[f'A{lay}_{g}'][:, n:n + 1])
                b_bc = sb.tile([128, HC], F16, tag="bbc", name="bbc")
                dma(b_bc[:], brow_dram.ap()[n:n + 1, c0:c0 + HC]
                    .broadcast_to((128, HC)))
                c_bc = sb.tile([128, HC], F16, tag="cbc", name="cbc")
                dma(c_bc[:], crow_dram.ap()[n:n + 1, c0:c0 + HC]
                    .broadcast_to((128, HC)))
                dBu = sb.tile([128, HC], F16, tag="dbu", name="dbu")
                nc.vector.tensor_tensor(out=dBu[:], in0=w16[:], in1=b_bc[:],
                                        op=OP.mult)
                if prev is not None:
                    _emit_q(*prev)
                h = sb.tile([128, HC], F16, tag="h", name="h")
                init = 0.0 if c == 0 else hlast[g][:, n:n + 1]
                nc.vector.tensor_tensor_scan(h[:], dA[:], dBu[:], init,
                                             OP.mult, OP.add)
                if c == 0:
                    nc.vector.tensor_copy(hlast[g][:, n:n + 1], h[:, -1:])
                prev = (n, h, c_bc)
            _emit_q(*prev)
            m_raw = sb.tile([128, HC], F16, tag=f"mraw_{g}", name="mraw") \
                if True else None
            for bi, (s, e) in enumerate(cols):
                t1 = sb.tile([128, BLK], F32, tag="gt1", name="gt1")
                nc.vector.scalar_tensor_tensor(
                    t1[:, :e - s], xc2[g][:, s:e], W[f'D{lay}_{g}'][:],
                    ytiles[bi][:, :e - s], OP.mult, OP.add)
                nc.vector.tensor_tensor(out=m_raw[:, s - c0:e - c0],
                                        in0=t1[:, :e - s],
                                        in1=sres[g][:, s:e], op=OP.mult)
            if g == 0:
                _mamba.mraws = []
            _mamba.mraws.append(m_raw)
        # out_proj over groups, then LN on this chunk
        for (s, e) in cols:
            mm = mmp.tile([128, BLK], F32, tag="mm", name="mm")
            for g in range(n_grp):
                nc.tensor.matmul(mm[:, :e - s], W[f'{n_wo_grp}{lay}_{g}'][:],
                                 _mamba.mraws[g][:, s - c0:e - c0],
                                 start=(g == 0), stop=(g == n_grp - 1))
            nc.scalar.activation(o_raw[:, s:e], mm[:, :e - s], AF.Copy)
        _layer_norm(nc, pools, dma, lnrows_dram, ln_row,
                    o_raw, g_col, b_col, out_norm,
                    out_off=out_norm_off, col0=c0, ncols=HC)


def build_nc():
    nc = bacc.Bacc("TRN2", target_bir_lowering=False, debug=False)
    dram_w = {}

    def reg(name, shape, dt):
        dram_w[name] = nc.dram_tensor(name, list(shape), dt,
                                      kind="ExternalInput")

    x_d = nc.dram_tensor("x_t", [IN_C, LPAD], F16, kind="ExternalInput")
    out_d = nc.dram_tensor("out", [128, 2048], F32, kind="ExternalOutput")
    reg('w64', [64, 6 * 128], F16)
    reg('w128', [128, 14 * 128], F16)
    reg('wxp', [128, 116], F16)
    reg('wdt', [8, 384], F32)
    reg('wA', [128, 48], F32)
    reg('wcols', [128, 17], F32)
    reg('ones128', [128, 1], F16)

    brow1 = nc.dram_tensor("brow1", [16, LC], F16)
    crow1 = nc.dram_tensor("crow1", [16, LC], F16)
    brow2 = nc.dram_tensor("brow2", [16, LC], F16)
    crow2 = nc.dram_tensor("crow2", [16, LC], F16)
    lnrows = nc.dram_tensor("lnrows", [6, LC], F16)

    with tile.TileContext(nc) as tc, ExitStack() as ctx:
        sb1 = ctx.enter_context(tc.tile_pool(name="sb1", bufs=1))
        sb2 = ctx.enter_context(tc.tile_pool(name="sb2", bufs=2))
        mmp = ctx.enter_context(tc.tile_pool(name="mmp", bufs=4,
                                             space="PSUM"))
        yp = ctx.enter_context(tc.tile_pool(name="yp", bufs=3,
                                            space="PSUM"))
        vec = ctx.enter_context(tc.tile_pool(name="vec", bufs=1))
        wpool = ctx.enter_context(tc.tile_pool(name="wp", bufs=1))
        dma = _DmaRR(nc)

        # x first; packed weights in a handful of DMAs on the scalar queue
        xt = wpool.tile([IN_C, LPAD], F16, tag="xt")
        nc.sync.dma_start(xt[:], x_d.ap())
        packed = {}
        for name in ['w64', 'wdt', 'wxp', 'wcols', 'wA', 'w128', 'ones128']:
            t = dram_w[name]
            tl = wpool.tile(list(t.shape), t.dtype, tag=name, name=name)
            dma.wload(tl[:], t.ap())
            packed[name] = tl
        w64, w128 = packed['w64'], packed['w128']
        Wt = {'ones128': packed['ones128']}
        for k, nm in enumerate(['Mj1_0_0', 'Mj1_1_0', 'Mj1_2_0', 'Mj1_3_0',
                                'Wres1_0', 'WlinT']):
            Wt[nm] = w64[:, k * 128:(k + 1) * 128]
        for k, nm in enumerate(['Mj2_0_0', 'Mj2_1_0', 'Mj2_2_0', 'Mj2_3_0',
                                'Mj2_0_1', 'Mj2_1_1', 'Mj2_2_1', 'Mj2_3_1',
                                'Wres2_0', 'Wres2_1', 'WoT1_0', 'WoT2_0',
                                'WoT2_1', 'ident']):
            Wt[nm] = w128[:, k * 128:(k + 1) * 128]
        Wt['xpT1_0'] = packed['wxp'][:, 0:36]
        Wt['xpT2_0'] = packed['wxp'][:, 36:76]
        Wt['xpT2_1'] = packed['wxp'][:, 76:116]
        Wt['dtwT1_0'] = packed['wdt'][0:4, 0:128]
        Wt['dtwT2_0'] = packed['wdt'][:, 128:256]
        Wt['dtwT2_1'] = packed['wdt'][:, 256:384]
        Wt['A1_0'] = packed['wA'][:, 0:16]
        Wt['A2_0'] = packed['wA'][:, 16:32]
        Wt['A2_1'] = packed['wA'][:, 32:48]
        for k, nm in enumerate(['convb1_0', 'dtb1_0', 'D1_0', 'convb2_0',
                                'convb2_1', 'dtb2_0', 'dtb2_1', 'D2_0',
                                'D2_1', 'ln1_g', 'ln1_b', 'ln2_g', 'ln2_b',
                                'lnr_g', 'lnr_b', 'linb', 'eps1']):
            Wt[nm] = packed['wcols'][:, k:k + 1]
        class _SbMux:
            P1 = {"xc_0", "xc_1", "sres_0", "sres_1", "r_n", "h1n",
                  "h2n", "hlast_0", "hlast_1", "lnbc0", "lnbc1",
                  "lnsq", "lnt1"}

            B3 = {"bbc", "cbc"}

            def tile(self, shape, dt, tag, name=None):
                pool = sb1 if tag in self.P1 else sb2
                bufs = 3 if tag in self.B3 else None
                return pool.tile(shape, dt, tag=tag, name=name or tag,
                                 bufs=bufs)

        pools = dict(sb=_SbMux(), mm=mmp, yacc=yp, vec=vec,
                     ident=Wt['ident'], ones128=Wt['ones128'],
                     eps1=Wt['eps1'])

        # mamba 1 (includes out_proj + LN1)
        h1n = pools["sb"].tile([128, LC + 3], F16, tag="h1n")
        nc.vector.memset(h1n[:, 0:3], 0.0)
        _mamba(nc, pools, dma, Wt, 1, xt, 3, 128, 4, brow1, crow1, 'WoT',
               lnrows, 2, Wt['ln1_g'], Wt['ln1_b'], h1n, 3)

        # mamba 2
        h2n = pools["sb"].tile([128, LC], F16, tag="h2n")
        _mamba(nc, pools, dma, Wt, 2, h1n, 3, 256, 8, brow2, crow2, 'WoT',
               lnrows, 4, Wt['ln2_g'], Wt['ln2_b'], h2n, 0)

        # residual linear branch (emitted last; fills the pipeline tail)
        r_raw = pools["sb"].tile([128, LC], F16, tag="rawbuf")
        for (s, e) in _blocks(LC):
            mm = mmp.tile([128, BLK], F32, tag="mm", name="mm")
            nc.tensor.matmul(mm[:, :e - s], Wt['WlinT'][:],
                             xt[:, 3 + s: 3 + e], start=True, stop=True)
            nc.scalar.activation(r_raw[:, s:e], mm[:, :e - s], AF.Identity,
                                 bias=Wt['linb'][:])
        r_n = pools["sb"].tile([128, LC], F16, tag="r_n")
        _layer_norm(nc, pools, dma, lnrows, 0, r_raw,
                    Wt['lnr_g'], Wt['lnr_b'], r_n)

        # final: out = r + h2n (last 2048 columns)
        for (s, e) in _blocks(2048):
            fin = pools["sb"].tile([128, BLK], F32, tag="fin")
            nc.vector.tensor_tensor(out=fin[:, :e - s],
                                    in0=r_n[:, OV + s:OV + e],
                                    in1=h2n[:, OV + s:OV + e], op=OP.add)
            dma(out_d.ap()[:, s:e], fin[:, :e - s])

    nc.compile()
    return nc


_NC_CACHE = {}


def _get_nc():
    if 'nc' not in _NC_CACHE:
        _NC_CACHE['nc'] = build_nc()
    return _NC_CACHE['nc']


def _host_weights(params):
    out = {}
    for lay, key, di in ((1, 'm1', 128), (2, 'm2', 256)):
        P = params[key]
        Win = np.asarray(P['in_proj'])          # [2di, dm]
        convw = np.asarray(P['conv_w'])         # [di, 4]
        for g in range(di // 128):
            rows = slice(g * 128, (g + 1) * 128)
            for j in range(D_CONV):
                Mj = (convw[rows, j:j + 1] * Win[:di][rows]).T
                out[f'Mj{lay}_{j}_{g}'] = np.ascontiguousarray(Mj).astype(np.float16)
            out[f'Wres{lay}_{g}'] = np.ascontiguousarray(
                Win[di:][rows].T).astype(np.float16)
            out[f'convb{lay}_{g}'] = np.asarray(P['conv_b'])[rows, None].astype(np.float32)
            out[f'xpT{lay}_{g}'] = np.ascontiguousarray(
                np.asarray(P['x_proj'])[:, rows].T).astype(np.float16)
            out[f'dtwT{lay}_{g}'] = np.ascontiguousarray(
                np.asarray(P['dt_w'])[rows].T).astype(np.float32)
            out[f'dtb{lay}_{g}'] = np.asarray(P['dt_b'])[rows, None].astype(np.float32)
            out[f'A{lay}_{g}'] = np.ascontiguousarray(
                -np.exp(np.asarray(P['A_log'])[rows])).astype(np.float32)
            out[f'D{lay}_{g}'] = np.asarray(P['D'])[rows, None].astype(np.float32)
            out[f'WoT{lay}_{g}'] = np.ascontiguousarray(
                np.asarray(P['out_proj'])[:, rows].T).astype(np.float16)
    out['ln1_g'] = np.asarray(params['ln1_g'])[:, None].astype(np.float32)
    out['ln1_b'] = np.asarray(params['ln1_b'])[:, None].astype(np.float32)
    out['ln2_g'] = np.asarray(params['ln2_g'])[:, None].astype(np.float32)
    out['ln2_b'] = np.asarray(params['ln2_b'])[:, None].astype(np.float32)
    out['WlinT'] = np.ascontiguousarray(
        np.asarray(params['lin_w']).T).astype(np.float16)
    out['linb'] = np.asarray(params['lin_b'])[:, None].astype(np.float32)
    out['lnr_g'] = np.asarray(params['ln_r_g'])[:, None].astype(np.float32)
    out['lnr_b'] = np.asarray(params['ln_r_b'])[:, None].astype(np.float32)
    out['ident'] = np.eye(128, dtype=np.float16)
    out['eps1'] = np.full((128, 1), 1e-5, np.float32)

    # pack into the device's fused input tensors
    packed = {}
    packed['w64'] = np.concatenate(
        [out[n] for n in ['Mj1_0_0', 'Mj1_1_0', 'Mj1_2_0', 'Mj1_3_0',
                          'Wres1_0', 'WlinT']], axis=1)
    packed['w128'] = np.concatenate(
        [out[n] for n in ['Mj2_0_0', 'Mj2_1_0', 'Mj2_2_0', 'Mj2_3_0',
                          'Mj2_0_1', 'Mj2_1_1', 'Mj2_2_1', 'Mj2_3_1',
                          'Wres2_0', 'Wres2_1', 'WoT1_0', 'WoT2_0',
                          'WoT2_1', 'ident']], axis=1)
    packed['wxp'] = np.concatenate(
        [out['xpT1_0'], out['xpT2_0'], out['xpT2_1']], axis=1)
    wdt = np.zeros((8, 384), np.float32)
    wdt[0:4, 0:128] = out['dtwT1_0']
    wdt[:, 128:256] = out['dtwT2_0']
    wdt[:, 256:384] = out['dtwT2_1']
    packed['wdt'] = wdt
    packed['wA'] = np.concatenate(
        [out['A1_0'], out['A2_0'], out['A2_1']], axis=1)
    packed['wcols'] = np.concatenate(
        [out[n] for n in ['convb1_0', 'dtb1_0', 'D1_0', 'convb2_0',
                          'convb2_1', 'dtb2_0', 'dtb2_1', 'D2_0', 'D2_1',
                          'ln1_g', 'ln1_b', 'ln2_g', 'ln2_b', 'lnr_g',
                          'lnr_b', 'linb', 'eps1']], axis=1)
    packed['ones128'] = np.full((128, 1), 1.0 / 128, np.float16)
    return packed


def kernel(x, params):
    x = np.asarray(x, np.float32)
    nc = _get_nc()
    wts = _host_weights(params)
    in_maps = []
    for c in range(8):
        b, half = c // 2, c % 2
        start = half * 2048 - OV
        xs = np.zeros((IN_C, LPAD), np.float16)
        lo = start - 3
        src_lo = max(lo, 0)
        src_hi = start + LC
        xs[:, src_lo - lo: src_lo - lo + (src_hi - src_lo)] = \
            x[b, src_lo:src_hi, :].T.astype(np.float16)
        m = dict(wts)
        m['x_t'] = xs
        in_maps.append(m)
    res = run_bass_kernel_spmd(nc, in_maps, core_ids=list(range(8)))
    _NC_CACHE['last_result'] = res
    out = np.zeros((B, L, 128), np.float32)
    for c in range(8):
        b, half = c // 2, c % 2
        out[b, half * 2048:(half + 1) * 2048, :] = res.results[c]['out'].T
    return out


# revision 25
# speedup vs baseline: 1.0938x; 1.0458x over previous
"""Trainium2 Bass kernel for the double-Mamba block (nn_DoubleConv).

Sharding: 8 cores = 4 batches x 2 sequence halves. Each core processes
LC = 2048 + OV columns of its batch element; OV columns are burn-in
(delta >= 0.54 => per-step state decay <= e^-0.54, so OV/2 columns per
mamba layer push the truncation error below fp32 noise).

Layout: features on partitions, time on the free axis.
 - causal depthwise conv folded into in_proj: 4 accumulating PE matmuls
   with shifted rhs, lhsT_j = (conv_w[:, j] * W_in_xc).T
 - dA_n = Exp(delta * A[:, n]) on ScalarE (per-partition scale AP)
 - h_n via VectorE tensor_tensor_scan (fp32 state)
 - B/C rows broadcast across partitions via DRAM round-trip DMAs
 - y = sum_n C_n*h_n via PE identity-matmul PSUM accumulation
 - LayerNorm over the feature(partition) axis via ones/128 matmuls
"""
import numpy as np
from contextlib import ExitStack

import concourse.bass as bass
import concourse.bacc as bacc
import concourse.mybir as mybir
import concourse.tile as tile
from concourse.bass_utils import run_bass_kernel_spmd

F32 = mybir.dt.float32
F16 = mybir.dt.float16
AF = mybir.ActivationFunctionType
OP = mybir.AluOpType

D_STATE = 16
D_CONV = 4
B, L, IN_C, OUT_C = 4, 4096, 64, 128
OV = 128                      # burn-in columns (covers both layers)
LC = 2048 + OV                # per-core columns
LPAD = LC + 3                 # conv left-pad
BLK = 512                     # PSUM block
N_CHUNKS = 1                  # time-chunks per layer


def _blocks(n, bs=BLK):
    return [(s, min(s + bs, n)) for s in range(0, n, bs)]


class _DmaRR:
    """DMA issue router: bulk broadcasts alternate Sync/GpSimd queues so
    the Scalar sequencer stays free for ACTIVATE dispatch."""

    def __init__(self, nc):
        self.nc = nc
        self.i = 0

    def __call__(self, out, in_):
        e = [self.nc.sync, self.nc.gpsimd][self.i % 2]
        self.i += 1
        return e.dma_start(out, in_)

    def wload(self, out, in_):
        return self.nc.scalar.dma_start(out, in_)


def _layer_norm(nc, pools, dma, lnrows_dram, row_base,
                h_raw, g_col, b_col, out_tile, out_off=0, col0=0, ncols=LC):
    """LN over the partition axis of h_raw[:, col0:col0+ncols] (f16, SBUF).
    Writes f16 into out_tile[:, out_off+col0 : out_off+col0+ncols]."""
    sb, mmp, vec = pools['sb'], pools['mm'], pools['vec']
    ones_over = pools['ones128']     # [128, 1] f16 of 1/128
    c1 = col0 + ncols
    h_sq = sb.tile([128, LC], F16, tag="lnsq", name="lnsq")
    nc.scalar.activation(h_sq[:, col0:c1], h_raw[:, col0:c1], AF.Square)
    vA = vec.tile([128, LC], F32, tag="vA", name="vA")
    vB = vec.tile([128, LC], F32, tag="vB", name="vB")
    msq, var, mu = vA[0:1, :], vA[32:33, :], vA[64:65, :]
    mu2, s_row = vB[0:1, :], vB[64:65, :]
    mus_row = vA[96:97, :]
    for (s, e) in _blocks(ncols):
        s, e = s + col0, e + col0
        p1 = mmp.tile([1, BLK], F32, tag="mm", name="mm")
        nc.tensor.matmul(p1[:, :e - s], ones_over[:], h_raw[:, s:e],
                         start=True, stop=True)
        nc.scalar.activation(mu[:, s:e], p1[:, :e - s], AF.Copy)
        p2 = mmp.tile([1, BLK], F32, tag="mm", name="mm")
        nc.tensor.matmul(p2[:, :e - s], ones_over[:], h_sq[:, s:e],
                         start=True, stop=True)
        nc.scalar.activation(msq[:, s:e], p2[:, :e - s], AF.Copy)
    nc.scalar.activation(mu2[:, col0:c1], mu[:, col0:c1], AF.Square)
    nc.vector.tensor_tensor(out=var[:, col0:c1], in0=msq[:, col0:c1],
                            in1=mu2[:, col0:c1], op=OP.subtract)
    nc.scalar.activation(var[:, col0:c1], var[:, col0:c1], AF.Ln,
                         bias=pools['eps1'][:1, :])
    nc.scalar.activation(s_row[:, col0:c1], var[:, col0:c1], AF.Exp,
                         scale=-0.5)
    nc.vector.tensor_tensor(out=mus_row[:, col0:c1], in0=mu[:, col0:c1],
                            in1=s_row[:, col0:c1], op=OP.mult)
    # f32 -> f16 cast happens inside the gpsimd software-DGE DMA
    nc.gpsimd.dma_start(out=lnrows_dram.ap()[row_base:row_base + 1, col0:c1],
                        in_=s_row[:, col0:c1])
    nc.gpsimd.dma_start(
        out=lnrows_dram.ap()[row_base + 1:row_base + 2, col0:c1],
        in_=mus_row[:, col0:c1])
    s_bc = sb.tile([128, LC], F16, tag="lnbc0", name="lnbc0")
    mus_bc = sb.tile([128, LC], F16, tag="lnbc1", name="lnbc1")
    dma(s_bc[:, col0:c1], lnrows_dram.ap()[row_base:row_base + 1, col0:c1]
        .broadcast_to((128, ncols)))
    dma(mus_bc[:, col0:c1],
        lnrows_dram.ap()[row_base + 1:row_base + 2, col0:c1]
        .broadcast_to((128, ncols)))
    # out = ((h*s_bc) - mus_bc)*g + b
    t1 = sb.tile([128, LC], F16, tag="lnt1", name="lnt1")
    nc.vector.tensor_tensor(out=t1[:, col0:c1], in0=h_raw[:, col0:c1],
                            in1=s_bc[:, col0:c1], op=OP.mult)
    t2 = sb.tile([128, LC], F16, tag="lnsq", name="lnsq")
    nc.vector.tensor_tensor(out=t2[:, col0:c1], in0=t1[:, col0:c1],
                            in1=mus_bc[:, col0:c1], op=OP.subtract)
    nc.vector.tensor_scalar(out=out_tile[:, out_off + col0:out_off + c1],
                            in0=t2[:, col0:c1], scalar1=g_col[:],
                            scalar2=b_col[:], op0=OP.mult, op1=OP.add)


def _mamba(nc, pools, dma, W, lay, xin, xin_off, di, dtr,
           brow_dram, crow_dram, n_wo_grp, lnrows_dram, ln_row, g_col,
           b_col, out_norm, out_norm_off):
    """One mamba layer, processed in 2 time-chunks so chunk-1 prep overlaps
    chunk-0 scans. Includes out_proj and LayerNorm. Writes normalized f16
    into out_norm[:, out_norm_off : out_norm_off+LC]."""
    sb, mmp, yp = pools['sb'], pools['mm'], pools['yacc']
    n_grp = di // 128
    HC = LC // N_CHUNKS
    hlast = [sb.tile([128, 16], F16, tag=f"hlast_{g}", name="hlast")
             for g in range(n_grp)]
    o_raw = sb.tile([128, LC], F16, tag="rawbuf", name="rawbuf")
    for c in range(N_CHUNKS):
        c0 = c * HC
        cols = [(c0 + s_, c0 + e_) for (s_, e_) in _blocks(HC)]
        xc2 = [sb.tile([128, LC], F16, tag=f"xc_{g}", name="xc")
               for g in range(n_grp)] if c == 0 else _mamba.xc2
        sres = [sb.tile([128, LC], F16, tag=f"sres_{g}", name="sres")
                for g in range(n_grp)] if c == 0 else _mamba.sres
        if c == 0:
            _mamba.xc2, _mamba.sres = xc2, sres
        for g in range(n_grp):
            for (s, e) in cols:
                mm = mmp.tile([128, BLK], F32, tag="mm", name="mm")
                for j in range(D_CONV):
                    nc.tensor.matmul(
                        mm[:, :e - s], W[f'Mj{lay}_{j}_{g}'][:],
                        xin[:, xin_off - 3 + j + s: xin_off - 3 + j + e],
                        start=(j == 0), stop=(j == D_CONV - 1))
                nc.scalar.activation(xc2[g][:, s:e], mm[:, :e - s], AF.Silu,
                                     bias=W[f'convb{lay}_{g}'][:])
                mm2 = mmp.tile([128, BLK], F32, tag="mm", name="mm")
                nc.tensor.matmul(mm2[:, :e - s], W[f'Wres{lay}_{g}'][:],
                                 xin[:, xin_off + s: xin_off + e],
                                 start=True, stop=True)
                nc.scalar.activation(sres[g][:, s:e], mm2[:, :e - s], AF.Silu)
        nxd = dtr + 32
        xdbl16 = sb.tile([nxd, LC], F16, tag="xdbl", name="xdbl") \
            if c == 0 else _mamba.xdbl
        dt32 = sb.tile([dtr, LC], F32, tag="dt32", name="dt32") \
            if c == 0 else _mamba.dt32
        if c == 0:
            _mamba.xdbl, _mamba.dt32 = xdbl16, dt32
        for (s, e) in cols:
            mm = mmp.tile([nxd, BLK], F32, tag="mm", name="mm")
            for g in range(n_grp):
                nc.tensor.matmul(mm[:, :e - s], W[f'xpT{lay}_{g}'][:],
                                 xc2[g][:, s:e],
                                 start=(g == 0), stop=(g == n_grp - 1))
            nc.scalar.activation(xdbl16[:, s:e], mm[:, :e - s], AF.Copy)
            nc.scalar.activation(dt32[:, s:e], mm[:dtr, :e - s], AF.Copy)
        dma(brow_dram.ap()[:, c0:c0 + HC], xdbl16[dtr:dtr + 16, c0:c0 + HC])
        dma(crow_dram.ap()[:, c0:c0 + HC],
            xdbl16[dtr + 16:dtr + 32, c0:c0 + HC])
        for g in range(n_grp):
            # delta = softplus(pre + dt_b) = ln(1 + exp(pre + dt_b))
            delta = sb.tile([128, HC], F32, tag="delta", name="delta")
            sigout = sb.tile([128, HC], F32, tag="sigout", name="sigout")
            for (s, e) in cols:
                mm = mmp.tile([128, BLK], F32, tag="mm", name="mm")
                nc.tensor.matmul(mm[:, :e - s], W[f'dtwT{lay}_{g}'][:],
                                 dt32[:, s:e], start=True, stop=True)
                nc.scalar.activation(sigout[:, s - c0:e - c0], mm[:, :e - s],
                                     AF.Exp, bias=W[f'dtb{lay}_{g}'][:])
            nc.scalar.activation(delta[:], sigout[:], AF.Ln, bias=1.0)
            w16 = sb.tile([128, HC], F16, tag="w16", name="w16")
            nc.vector.tensor_tensor(out=w16[:], in0=delta[:],
                                    in1=xc2[g][:, c0:c0 + HC], op=OP.mult)
            ytiles = [yp.tile([128, BLK], F32, tag="yacc", name="yacc")
                      for _ in cols]

            def _emit_q(n, h, c_bc):
                q = sb.tile([128, HC], F16, tag="q", name="q")
                nc.vector.tensor_tensor(out=q[:], in0=h[:], in1=c_bc[:],
                                        op=OP.mult)
                for bi, (s, e) in enumerate(cols):
                    nc.tensor.matmul(ytiles[bi][:, :e - s], pools['ident'][:],
                                     q[:, s - c0:e - c0],
                                     start=(n == 0), stop=(n == 15))

            prev = None
            for n in range(16):
                dA = sb.tile([128, HC], F32, tag="dA", name="dA")
                nc.scalar.activation(dA[:], delta[:], AF.Exp,
                                     scale=W[f'A{lay}_{g}'][:, n:n + 1])
                b_bc = sb.tile([128, HC], F16, tag="bbc", name="bbc")
                dma(b_bc[:], brow_dram.ap()[n:n + 1, c0:c0 + HC]
                    .broadcast_to((128, HC)))
                c_bc = sb.tile([128, HC], F16, tag="cbc", name="cbc")
                dma(c_bc[:], crow_dram.ap()[n:n + 1, c0:c0 + HC]
                    .broadcast_to((128, HC)))
                dBu = sb.tile([128, HC], F16, tag="dbu", name="dbu")
                nc.vector.tensor_tensor(out=dBu[:], in0=w16[:], in1=b_bc[:],
                                        op=OP.mult)
                if prev is not None:
                    _emit_q(*prev)
                h = sb.tile([128, HC], F16, tag="h", name="h")
                init = 0.0 if c == 0 else hlast[g][:, n:n + 1]
                nc.vector.tensor_tensor_scan(h[:], dA[:], dBu[:], init,
                                             OP.mult, OP.add)
                if c == 0 and N_CHUNKS > 1:
                    nc.vector.tensor_copy(hlast[g][:, n:n + 1], h[:, -1:])
                prev = (n, h, c_bc)
            _emit_q(*prev)
            m_raw = sb.tile([128, HC], F16, tag=f"mraw_{g}", name="mraw") \
                if True else None
            for bi, (s, e) in enumerate(cols):
                t1 = sb.tile([128, BLK], F32, tag="gt1", name="gt1")
                nc.vector.scalar_tensor_tensor(
                    t1[:, :e - s], xc2[g][:, s:e], W[f'D{lay}_{g}'][:],
                    ytiles[bi][:, :e - s], OP.mult, OP.add)
                nc.vector.tensor_tensor(out=m_raw[:, s - c0:e - c0],
                                        in0=t1[:, :e - s],
                                        in1=sres[g][:, s:e], op=OP.mult)
            if g == 0:
                _mamba.mraws = []
            _mamba.mraws.append(m_raw)
        # out_proj over groups, then LN on this chunk
        for (s, e) in cols:
            mm = mmp.tile([128, BLK], F32, tag="mm", name="mm")
            for g in range(n_grp):
                nc.tensor.matmul(mm[:, :e - s], W[f'{n_wo_grp}{lay}_{g}'][:],
                                 _mamba.mraws[g][:, s - c0:e - c0],
                                 start=(g == 0), stop=(g == n_grp - 1))
            nc.scalar.activation(o_raw[:, s:e], mm[:, :e - s], AF.Copy)
        _layer_norm(nc, pools, dma, lnrows_dram, ln_row,
                    o_raw, g_col, b_col, out_norm,
                    out_off=out_norm_off, col0=c0, ncols=HC)


def build_nc():
    nc = bacc.Bacc("TRN2", target_bir_lowering=False, debug=False)
    dram_w = {}

    def reg(name, shape, dt):
        dram_w[name] = nc.dram_tensor(name, list(shape), dt,
                                      kind="ExternalInput")

    x_d = nc.dram_tensor("x_t", [IN_C, LPAD], F16, kind="ExternalInput")
    out_d = nc.dram_tensor("out", [128, 2048], F32, kind="ExternalOutput")
    reg('w64', [64, 6 * 128], F16)
    reg('w128', [128, 14 * 128], F16)
    reg('wxp', [128, 116], F16)
    reg('wdt', [8, 384], F32)
    reg('wA', [128, 48], F32)
    reg('wcols', [128, 17], F32)
    reg('ones128', [128, 1], F16)

    brow1 = nc.dram_tensor("brow1", [16, LC], F16)
    crow1 = nc.dram_tensor("crow1", [16, LC], F16)
    brow2 = nc.dram_tensor("brow2", [16, LC], F16)
    crow2 = nc.dram_tensor("crow2", [16, LC], F16)
    lnrows = nc.dram_tensor("lnrows", [6, LC], F16)

    with tile.TileContext(nc) as tc, ExitStack() as ctx:
        sb1 = ctx.enter_context(tc.tile_pool(name="sb1", bufs=1))
        sb2 = ctx.enter_context(tc.tile_pool(name="sb2", bufs=2))
        mmp = ctx.enter_context(tc.tile_pool(name="mmp", bufs=3,
                                             space="PSUM"))
        yp = ctx.enter_context(tc.tile_pool(name="yp", bufs=5,
                                            space="PSUM"))
        vec = ctx.enter_context(tc.tile_pool(name="vec", bufs=1))
        wpool = ctx.enter_context(tc.tile_pool(name="wp", bufs=1))
        dma = _DmaRR(nc)

        # x first; packed weights in a handful of DMAs on the scalar queue
        xt = wpool.tile([IN_C, LPAD], F16, tag="xt")
        nc.sync.dma_start(xt[:], x_d.ap())
        packed = {}
        for name in ['w64', 'wdt', 'wxp', 'wcols', 'wA', 'w128', 'ones128']:
            t = dram_w[name]
            tl = wpool.tile(list(t.shape), t.dtype, tag=name, name=name)
            dma.wload(tl[:], t.ap())
            packed[name] = tl
        w64, w128 = packed['w64'], packed['w128']
        Wt = {'ones128': packed['ones128']}
        for k, nm in enumerate(['Mj1_0_0', 'Mj1_1_0', 'Mj1_2_0', 'Mj1_3_0',
                                'Wres1_0', 'WlinT']):
            Wt[nm] = w64[:, k * 128:(k + 1) * 128]
        for k, nm in enumerate(['Mj2_0_0', 'Mj2_1_0', 'Mj2_2_0', 'Mj2_3_0',
                                'Mj2_0_1', 'Mj2_1_1', 'Mj2_2_1', 'Mj2_3_1',
                                'Wres2_0', 'Wres2_1', 'WoT1_0', 'WoT2_0',
                                'WoT2_1', 'ident']):
            Wt[nm] = w128[:, k * 128:(k + 1) * 128]
        Wt['xpT1_0'] = packed['wxp'][:, 0:36]
        Wt['xpT2_0'] = packed['wxp'][:, 36:76]
        Wt['xpT2_1'] = packed['wxp'][:, 76:116]
        Wt['dtwT1_0'] = packed['wdt'][0:4, 0:128]
        Wt['dtwT2_0'] = packed['wdt'][:, 128:256]
        Wt['dtwT2_1'] = packed['wdt'][:, 256:384]
        Wt['A1_0'] = packed['wA'][:, 0:16]
        Wt['A2_0'] = packed['wA'][:, 16:32]
        Wt['A2_1'] = packed['wA'][:, 32:48]
        for k, nm in enumerate(['convb1_0', 'dtb1_0', 'D1_0', 'convb2_0',
                                'convb2_1', 'dtb2_0', 'dtb2_1', 'D2_0',
                                'D2_1', 'ln1_g', 'ln1_b', 'ln2_g', 'ln2_b',
                                'lnr_g', 'lnr_b', 'linb', 'eps1']):
            Wt[nm] = packed['wcols'][:, k:k + 1]
        class _SbMux:
            P1 = {"xc_0", "xc_1", "sres_0", "sres_1", "r_n", "h1n",
                  "h2n", "hlast_0", "hlast_1", "lnbc0", "lnbc1",
                  "lnsq", "lnt1", "delta", "sigout", "w16", "xdbl",
                  "dt32"}

            B3 = set()

            def tile(self, shape, dt, tag, name=None):
                pool = sb1 if tag in self.P1 else sb2
                bufs = 3 if tag in self.B3 else None
                return pool.tile(shape, dt, tag=tag, name=name or tag,
                                 bufs=bufs)

        pools = dict(sb=_SbMux(), mm=mmp, yacc=yp, vec=vec,
                     ident=Wt['ident'], ones128=Wt['ones128'],
                     eps1=Wt['eps1'])

        # mamba 1 (includes out_proj + LN1)
        h1n = pools["sb"].tile([128, LC + 3], F16, tag="h1n")
        nc.vector.memset(h1n[:, 0:3], 0.0)
        _mamba(nc, pools, dma, Wt, 1, xt, 3, 128, 4, brow1, crow1, 'WoT',
               lnrows, 2, Wt['ln1_g'], Wt['ln1_b'], h1n, 3)

        # mamba 2
        h2n = pools["sb"].tile([128, LC], F16, tag="h2n")
        _mamba(nc, pools, dma, Wt, 2, h1n, 3, 256, 8, brow2, crow2, 'WoT',
               lnrows, 4, Wt['ln2_g'], Wt['ln2_b'], h2n, 0)

        # residual linear branch (emitted last; fills the pipeline tail)
        r_raw = pools["sb"].tile([128, LC], F16, tag="rawbuf")
        for (s, e) in _blocks(LC):
            mm = mmp.tile([128, BLK], F32, tag="mm", name="mm")
            nc.tensor.matmul(mm[:, :e - s], Wt['WlinT'][:],
                             xt[:, 3 + s: 3 + e], start=True, stop=True)
            nc.scalar.activation(r_raw[:, s:e], mm[:, :e - s], AF.Identity,
                                 bias=Wt['linb'][:])
        r_n = pools["sb"].tile([128, LC], F16, tag="r_n")
        _layer_norm(nc, pools, dma, lnrows, 0, r_raw,
                    Wt['lnr_g'], Wt['lnr_b'], r_n)

        # final: out = r + h2n (last 2048 columns)
        for (s, e) in _blocks(2048):
            fin = pools["sb"].tile([128, BLK], F32, tag="fin")
            nc.vector.tensor_tensor(out=fin[:, :e - s],
                                    in0=r_n[:, OV + s:OV + e],
                                    in1=h2n[:, OV + s:OV + e], op=OP.add)
            dma(out_d.ap()[:, s:e], fin[:, :e - s])

    nc.compile()
    return nc


_NC_CACHE = {}


def _get_nc():
    if 'nc' not in _NC_CACHE:
        _NC_CACHE['nc'] = build_nc()
    return _NC_CACHE['nc']


def _host_weights(params):
    out = {}
    for lay, key, di in ((1, 'm1', 128), (2, 'm2', 256)):
        P = params[key]
        Win = np.asarray(P['in_proj'])          # [2di, dm]
        convw = np.asarray(P['conv_w'])         # [di, 4]
        for g in range(di // 128):
            rows = slice(g * 128, (g + 1) * 128)
            for j in range(D_CONV):
                Mj = (convw[rows, j:j + 1] * Win[:di][rows]).T
                out[f'Mj{lay}_{j}_{g}'] = np.ascontiguousarray(Mj).astype(np.float16)
            out[f'Wres{lay}_{g}'] = np.ascontiguousarray(
                Win[di:][rows].T).astype(np.float16)
            out[f'convb{lay}_{g}'] = np.asarray(P['conv_b'])[rows, None].astype(np.float32)
            out[f'xpT{lay}_{g}'] = np.ascontiguousarray(
                np.asarray(P['x_proj'])[:, rows].T).astype(np.float16)
            out[f'dtwT{lay}_{g}'] = np.ascontiguousarray(
                np.asarray(P['dt_w'])[rows].T).astype(np.float32)
            out[f'dtb{lay}_{g}'] = np.asarray(P['dt_b'])[rows, None].astype(np.float32)
            out[f'A{lay}_{g}'] = np.ascontiguousarray(
                -np.exp(np.asarray(P['A_log'])[rows])).astype(np.float32)
            out[f'D{lay}_{g}'] = np.asarray(P['D'])[rows, None].astype(np.float32)
            out[f'WoT{lay}_{g}'] = np.ascontiguousarray(
                np.asarray(P['out_proj'])[:, rows].T).astype(np.float16)
    out['ln1_g'] = np.asarray(params['ln1_g'])[:, None].astype(np.float32)
    out['ln1_b'] = np.asarray(params['ln1_b'])[:, None].astype(np.float32)
    out['ln2_g'] = np.asarray(params['ln2_g'])[:, None].astype(np.float32)
    out['ln2_b'] = np.asarray(params['ln2_b'])[:, None].astype(np.float32)
    out['WlinT'] = np.ascontiguousarray(
        np.asarray(params['lin_w']).T).astype(np.float16)
    out['linb'] = np.asarray(params['lin_b'])[:, None].astype(np.float32)
    out['lnr_g'] = np.asarray(params['ln_r_g'])[:, None].astype(np.float32)
    out['lnr_b'] = np.asarray(params['ln_r_b'])[:, None].astype(np.float32)
    out['ident'] = np.eye(128, dtype=np.float16)
    out['eps1'] = np.full((128, 1), 1e-5, np.float32)

    # pack into the device's fused input tensors
    packed = {}
    packed['w64'] = np.concatenate(
        [out[n] for n in ['Mj1_0_0', 'Mj1_1_0', 'Mj1_2_0', 'Mj1_3_0',
                          'Wres1_0', 'WlinT']], axis=1)
    packed['w128'] = np.concatenate(
        [out[n] for n in ['Mj2_0_0', 'Mj2_1_0', 'Mj2_2_0', 'Mj2_3_0',
                          'Mj2_0_1', 'Mj2_1_1', 'Mj2_2_1', 'Mj2_3_1',
                          'Wres2_0', 'Wres2_1', 'WoT1_0', 'WoT2_0',
                          'WoT2_1', 'ident']], axis=1)
    packed['wxp'] = np.concatenate(
        [out['xpT1_0'], out['xpT2_0'], out['xpT2_1']], axis=1)
    wdt = np.zeros((8, 384), np.float32)
    wdt[0:4, 0:128] = out['dtwT1_0']
    wdt[:, 128:256] = out['dtwT2_0']
    wdt[:, 256:384] = out['dtwT2_1']
    packed['wdt'] = wdt
    packed['wA'] = np.concatenate(
        [out['A1_0'], out['A2_0'], out['A2_1']], axis=1)
    packed['wcols'] = np.concatenate(
        [out[n] for n in ['convb1_0', 'dtb1_0', 'D1_0', 'convb2_0',
                          'convb2_1', 'dtb2_0', 'dtb2_1', 'D2_0', 'D2_1',
                          'ln1_g', 'ln1_b', 'ln2_g', 'ln2_b', 'lnr_g',
                          'lnr_b', 'linb', 'eps1']], axis=1)
    packed['ones128'] = np.full((128, 1), 1.0 / 128, np.float16)
    return packed


def kernel(x, params):
    x = np.asarray(x, np.float32)
    nc = _get_nc()
    wts = _host_weights(params)
    in_maps = []
    for c in range(8):
        b, half = c // 2, c % 2
        start = half * 2048 - OV
        xs = np.zeros((IN_C, LPAD), np.float16)
        lo = start - 3
        src_lo = max(lo, 0)
        src_hi = start + LC
        xs[:, src_lo - lo: src_lo - lo + (src_hi - src_lo)] = \
            x[b, src_lo:src_hi, :].T.astype(np.float16)
        m = dict(wts)
        m['x_t'] = xs
        in_maps.append(m)
    res = run_bass_kernel_spmd(nc, in_maps, core_ids=list(range(8)))
    _NC_CACHE['last_result'] = res
    out = np.zeros((B, L, 128), np.float32)
    for c in range(8):
        b, half = c // 2, c % 2
        out[b, half * 2048:(half + 1) * 2048, :] = res.results[c]['out'].T
    return out
